# revision 8
# baseline (speedup 1.0000x reference)
"""Multi-head causal attention (B=8, T=2048, C=384, H=6, Dh=64) on 8 TRN2 cores.

Sharding: data-parallel over batch - core b computes batch element b end to end
(no collectives).

v4 layout (all "T" means transposed, head-dim/channel on partitions):
  xT   [128, 3, 2048]  bf16   c = 128*ci + p
  wq/wk[128, 3, 384]   bf16   packed Wq[h,c,d] -> [c, h*64+d]
  wv/wp[128, 3, 384]   bf16
  biasb[128, 384]      f32    bias replicated across partitions

Per-core compute:
  QT/KT [hd, t] via matmul; Vt [s, h, 65] augmented (col 64 == 1 -> denom row).
  Attention per (j q-block of 512, hp head-pair, i s-chunk of 128):
    S-pair [128, 1024] = two concurrent K=64 matmuls (tile rows 0:64 / 64:128)
    one wide exp (ACT) -> P bf16 [128, 1024]; causal diag via affine_select
    PV per head accumulates O[65, 512] (row 64 = softmax denominator)
  O tiles are double-buffered (bufs=2 per head) so the eviction chain of one
  frame never blocks the next frame's first PV.
  Denominator chain is split + deferred: denom rows -> (next frame i==1)
  reciprocal_approx_fast + fp16 cast -> (i==5) K=1 broadcast matmuls into a
  recipB region of the S rotation -> tensor_mul normalizes attT in place.
  Output projection is a dense K=128 accumulation Y[t,e] = attT^T @ wp
  (+bias via tensor_add on eviction), Y psum also in the S rotation.
  PSUM: 4 banks S-rotation (2x [128,1024]) + 4 banks O (2 heads x 2 bufs).
  Invariant: an even number of aux tile-calls between consecutive S-chunk
  calls keeps S double-buffering intact (pad tiles enforce it).
  Phase-1 projection tasks are schedule-driven (frame plan) with JIT fallback.
"""

import numpy as np
import ml_dtypes

import concourse.bass as bass
import concourse.tile as tile
from concourse import bacc, mybir
from concourse.bass import ts, ds

F32 = mybir.dt.float32
BF16 = mybir.dt.bfloat16
FP16 = mybir.dt.float16
AF = mybir.ActivationFunctionType

B, T, C = 8, 2048, 384
H, DH = 6, 64
SCALE = DH ** -0.5
NCORES = 8
TJ = 512            # q-block width
NJ = T // TJ        # 4 q-blocks
SC = 128            # s-chunk
NCI = C // 128      # 3 channel chunks
NHP = H // 2        # 3 head pairs (= hd blocks)


def build_kernel():
    nc = bacc.Bacc("TRN2", target_bir_lowering=False, debug=False)

    xT_d = nc.dram_tensor("xT", [128, NCI, T], BF16, kind="ExternalInput").ap()
    wq_d = nc.dram_tensor("wq", [128, NCI, C], BF16, kind="ExternalInput").ap()
    wk_d = nc.dram_tensor("wk", [128, NCI, C], BF16, kind="ExternalInput").ap()
    wv_d = nc.dram_tensor("wv", [128, NCI, C], BF16, kind="ExternalInput").ap()
    wp_d = nc.dram_tensor("wp", [128, NCI, C], BF16, kind="ExternalInput").ap()
    biasb_d = nc.dram_tensor("biasb", [128, C], F32, kind="ExternalInput").ap()
    y_d = nc.dram_tensor("y", [T, C], F32, kind="ExternalOutput").ap()

    with tile.TileContext(nc) as tc:
        with tc.tile_pool(name="const", bufs=1) as cpool, \
             tc.tile_pool(name="pp", bufs=2) as pp, \
             tc.tile_pool(name="sp", bufs=2, space="PSUM") as sp, \
             tc.tile_pool(name="op", bufs=2, space="PSUM") as op:
            xT = cpool.tile([128, NCI, T], BF16)
            wq = cpool.tile([128, NCI, C], BF16)
            wk = cpool.tile([128, NCI, C], BF16)
            wv = cpool.tile([128, NCI, C], BF16)
            wp = cpool.tile([128, NCI, C], BF16)
            biasb = cpool.tile([128, C], F32)
            QT = cpool.tile([128, NCI, T], BF16)
            KT = cpool.tile([128, NCI, T], BF16)
            attT = cpool.tile([128, NCI, T], BF16)
            Vt = cpool.tile([128, 16, H, 65], BF16)
            ones16 = cpool.tile([1, 64], FP16)
            scr = cpool.tile([1, 64], F32)

            # whole-tile memset (strided memset fails ISA check); V copies
            # overwrite cols 0:64 leaving col 64 == 1.0 (denominator trick)
            nc.gpsimd.memset(Vt[:], 1.0)
            nc.gpsimd.memset(ones16[:], 1.0)
            # preload the exp activation table while DMAs run
            nc.scalar.activation(scr[:], ones16[:], AF.Exp, scale=1.0)

            nc.sync.dma_start(xT[:, 0, :], xT_d[:, 0, :])
            nc.sync.dma_start(wk[:], wk_d[:])
            nc.sync.dma_start(wq[:], wq_d[:])
            nc.sync.dma_start(xT[:, 1, :], xT_d[:, 1, :])
            nc.sync.dma_start(xT[:, 2, :], xT_d[:, 2, :])
            nc.sync.dma_start(wv[:], wv_d[:])
            nc.sync.dma_start(wp[:], wp_d[:])
            nc.sync.dma_start(biasb[:], biasb_d[:])

            # ---- S-rotation bookkeeping: even aux calls between S chunks ----
            aux_since_S = [0]

            def aux_tile():
                aux_since_S[0] += 1
                t = sp.tile([128, 1024], F32, tag="S", name="aux")
                return t

            def pad_parity():
                if aux_since_S[0] % 2 == 1:
                    t = aux_tile()
                    nc.vector.memset(t[0:1, 0:1], 0.0)

            # zero-init both S psum buffers: wide exp calls read full tiles
            # and must never see boot garbage (NaN) even in unused columns
            for _ in range(2):
                z0 = aux_tile()
                nc.vector.memset(z0[:], 0.0)

            # ---- phase-1 projection tasks (share the S rotation) ----
            done = set()

            def proj_qk(dst, w, bi, q):  # one 512-col quarter
                ps = aux_tile()
                for ci in range(NCI):
                    nc.tensor.matmul(
                        ps[:, 0:TJ],
                        lhsT=w[:, ci, ts(bi, 128)],
                        rhs=xT[:, ci, ts(q, 512)],
                        start=(ci == 0), stop=(ci == NCI - 1),
                    )
                nc.vector.tensor_copy(dst[:, bi, ts(q, 512)], ps[:, 0:TJ])

            def proj_v(si):
                ps = aux_tile()
                for ci in range(NCI):
                    nc.tensor.matmul(
                        ps[:, 0:C],
                        lhsT=xT[:, ci, ts(si, 128)],
                        rhs=wv[:, ci, :],
                        start=(ci == 0), stop=(ci == NCI - 1),
                    )
                nc.vector.tensor_copy(
                    Vt[:, si, :, 0:64],
                    ps[:, 0:C].rearrange("p (h d) -> p h d", h=H),
                )

            def emit_key(key):
                if key in done:
                    return
                done.add(key)
                if key[0] == 'K':
                    proj_qk(KT, wk, key[1], key[2])
                elif key[0] == 'Q':
                    proj_qk(QT, wq, key[1], key[2])
                else:
                    proj_v(key[1])

            # prefix: exactly what (j=0, hp=0) starts with
            for key in [('K', 0, 0), ('Q', 0, 0), ('V', 0), ('V', 1)]:
                emit_key(key)

            # per-frame drip plan: frame index f = 3*j + hp -> list of pairs
            # consumed at chunk slots i == 2, 3, 4 (j0 frames: i == 2, 3)
            PLAN = {
                0: [[('V', 2), ('V', 3)], [('K', 1, 0), ('Q', 1, 0)]],
                1: [[('K', 2, 0), ('Q', 2, 0)]],
                2: [[('K', 0, 1), ('Q', 0, 1)]],
                3: [[('V', 4), ('V', 5)], [('V', 6), ('V', 7)],
                    [('Q', 1, 1), ('K', 1, 1)]],
                4: [[('K', 2, 1), ('Q', 2, 1)], [('V', 8), ('V', 9)],
                    [('K', 0, 2), ('Q', 0, 2)]],
                5: [[('V', 10), ('V', 11)], [('K', 1, 2), ('Q', 1, 2)],
                    [('K', 2, 2), ('Q', 2, 2)]],
                6: [[('Q', 0, 3), ('K', 0, 3)], [('V', 12), ('V', 13)],
                    [('V', 14), ('V', 15)]],
                7: [[('Q', 1, 3), ('K', 1, 3)], [('Q', 2, 3), ('K', 2, 3)]],
            }

            ytasks = []       # deferred output-projection closures
            norm_pre = []     # deferred recip+fp16 cast (DVE only)
            norm_post = []    # deferred broadcast+tensor_mul
            ndone = [0]       # completed normalizations

            def make_y(j_, tb_):
                def run():
                    Y = aux_tile()
                    for bi in range(NCI):
                        nc.tensor.matmul(
                            Y[:, 0:C],
                            lhsT=attT[:, bi, ts(4 * j_ + tb_, 128)],
                            rhs=wp[:, bi, :],
                            start=(bi == 0), stop=(bi == NCI - 1),
                        )
                    ysb = pp.tile([128, C], F32, tag="ysb")
                    nc.vector.tensor_add(ysb[:], Y[:, 0:C], biasb[:])
                    nc.sync.dma_start(y_d[ts(4 * j_ + tb_, 128), :], ysb[:])
                return run

            # ---- phase 2: attention ----
            for j in range(NJ):
                for hp in range(NHP):
                    f = 3 * j + hp
                    plan = PLAN.get(f, [])
                    ha, hb = 2 * hp, 2 * hp + 1
                    # JIT safety net for this frame's K quarters / Q block
                    for q in range(j + 1):
                        emit_key(('K', hp, q))
                    emit_key(('Q', hp, j))
                    Oa = op.tile([65, TJ], F32, tag="Oa")
                    Ob = op.tile([65, TJ], F32, tag="Ob")
                    nch = 4 * j + 4
                    Ps = [None] * nch

                    def emit_S(i):
                        d = max(0, SC * i - TJ * j)
                        pad_parity()
                        aux_since_S[0] = 0
                        st = sp.tile([128, 1024], F32, tag="S")
                        for z in (0, 64):
                            nc.tensor.matmul(
                                st[:, 8 * z + d:8 * z + 512],
                                lhsT=KT[z:z + 64, hp, ts(i, SC)],
                                rhs=QT[z:z + 64, hp, ds(TJ * j + d, TJ - d)],
                                start=True, stop=True,
                            )
                        P = pp.tile([128, 1024], BF16, tag="P")
                        if d <= 128:
                            nc.scalar.activation(P[:], st[:], AF.Exp, scale=SCALE)
                        else:
                            for z in (0, 512):
                                nc.scalar.activation(P[:, z + d:z + 512],
                                                     st[:, z + d:z + 512],
                                                     AF.Exp, scale=SCALE)
                        if SC * i >= TJ * j:  # fringe: mask diag window
                            for z in (0, 512):
                                nc.gpsimd.affine_select(
                                    out=P[:, z + d:z + d + 128],
                                    in_=P[:, z + d:z + d + 128],
                                    pattern=[[1, 128]],
                                    compare_op=mybir.AluOpType.is_ge,
                                    fill=0.0, base=0, channel_multiplier=-1,
                                )
                        Ps[i] = P

                    def emit_PV(i):
                        d = max(0, SC * i - TJ * j)
                        emit_key(('V', i))
                        for O, z, h in ((Oa, 0, ha), (Ob, 512, hb)):
                            nc.tensor.matmul(
                                O[:, d:TJ],
                                lhsT=Vt[:, i, h, :],
                                rhs=Ps[i][:, z + d:z + 512],
                                start=(i == 0), stop=(i == nch - 1),
                            )

                    norm_slot = 3 if j == 0 else 5
                    drip_slots = (2, 3, 4) if j > 0 else ((2, 3) if f == 0
                                                          else (2,))
                    pi = 0
                    for i in range(nch):
                        emit_S(i)
                        if i >= 1:
                            emit_PV(i - 1)
                        if i == 1 and norm_pre:
                            norm_pre.pop(0)()
                        if i in drip_slots and pi < len(plan):
                            for key in plan[pi]:
                                emit_key(key)
                            pi += 1
                        elif i == norm_slot and norm_post:
                            norm_post.pop(0)()
                        elif i >= 6 and i % 2 == 0 and ytasks and \
                                ndone[0] >= 3 * (ytasks[0][0] + 1):
                            for _ in range(2):
                                if ytasks and \
                                        ndone[0] >= 3 * (ytasks[0][0] + 1):
                                    ytasks.pop(0)[1]()
                    emit_PV(nch - 1)

                    # evict attT (unnormalized) + denominator rows, head b
                    # first (its O has no double-buffer slack to spare)
                    dsb = pp.tile([1, 2 * TJ], F32, tag="dsb")
                    nc.vector.tensor_copy(attT[64:128, hp, ts(j, TJ)], Ob[0:64, :])
                    nc.vector.tensor_copy(dsb[0:1, TJ:2 * TJ], Ob[64:65, :])
                    nc.vector.tensor_copy(attT[0:64, hp, ts(j, TJ)], Oa[0:64, :])
                    nc.vector.tensor_copy(dsb[0:1, 0:TJ], Oa[64:65, :])

                    def make_pre(dsb_):
                        r16 = pp.tile([1, 2 * TJ], FP16, tag="r16",
                                      name="r16")
                        def run():
                            rr = pp.tile([1, 2 * TJ], F32, tag="rr")
                            nc.vector.reciprocal_approx_fast(rr[:], dsb_[:])
                            nc.vector.tensor_copy(r16[:], rr[:])
                        return run, r16

                    def make_post(j_, hp_, r16_):
                        def run():
                            rB = aux_tile()
                            for z in (0, 64):
                                for q in range(4):
                                    nc.tensor.matmul(
                                        rB[z:z + 64, ts(q, 128)],
                                        lhsT=ones16[0:1, :],
                                        rhs=r16_[0:1, ds(8 * z + 128 * q, 128)],
                                        start=True, stop=True,
                                    )
                            nc.vector.tensor_mul(attT[:, hp_, ts(j_, TJ)],
                                                 attT[:, hp_, ts(j_, TJ)],
                                                 rB[:, 0:TJ])
                            ndone[0] += 1
                        return run
                    pre, r16h = make_pre(dsb)
                    norm_pre.append(pre)
                    norm_post.append(make_post(j, hp, r16h))

                for tb in range(4):
                    ytasks.append((j, make_y(j, tb)))

            while norm_pre:   # tail: last normalize + j=3 output blocks
                norm_pre.pop(0)()
            while norm_post:
                norm_post.pop(0)()
            while ytasks:
                ytasks.pop(0)[1]()

    nc.compile()
    return nc


def _prep_inputs(x, Wq, Wk, Wv, Wp, bp):
    """Host-side shard + layout prep. Returns per-core input maps."""
    bf = ml_dtypes.bfloat16
    x = np.asarray(x, dtype=np.float32)

    def pack_w(W):  # [H, C, Dh] -> [128, NCI, H*Dh]
        Whd = np.transpose(np.asarray(W, np.float32), (1, 0, 2)).reshape(C, H * DH)
        return np.ascontiguousarray(
            Whd.reshape(NCI, 128, H * DH).transpose(1, 0, 2)
        ).astype(bf)

    wq_p, wk_p, wv_p = pack_w(Wq), pack_w(Wk), pack_w(Wv)
    wp_p = np.ascontiguousarray(
        np.asarray(Wp, np.float32).reshape(NCI, 128, C).transpose(1, 0, 2)
    ).astype(bf)
    biasb = np.broadcast_to(np.asarray(bp, np.float32), (128, C)).copy()

    in_maps = []
    for b in range(B):
        xT = np.ascontiguousarray(
            x[b].T.reshape(NCI, 128, T).transpose(1, 0, 2)
        ).astype(bf)
        in_maps.append({
            "xT": xT, "wq": wq_p, "wk": wk_p, "wv": wv_p, "wp": wp_p,
            "biasb": biasb,
        })
    return in_maps


_CACHE = {}


def kernel(x, Wq, Wk, Wv, Wp, bp):
    from concourse.bass_utils import run_bass_kernel_spmd

    if "nc" not in _CACHE:
        _CACHE["nc"] = build_kernel()
    nc = _CACHE["nc"]
    in_maps = _prep_inputs(x, Wq, Wk, Wv, Wp, bp)
    res = run_bass_kernel_spmd(nc, in_maps, list(range(NCORES)))
    out = np.stack([res.results[b]["y"] for b in range(B)], axis=0)
    return out.astype(np.float32)


# revision 9
# speedup vs baseline: 1.1486x; 1.1486x over previous
"""Multi-head causal attention (B=8, T=2048, C=384, H=6, Dh=64) on 8 TRN2 cores.

Sharding: data-parallel over batch - core b computes batch element b end to end
(no collectives).

v5 layout (all "T" means transposed, head-dim/channel on partitions):
  xT   [128, 3, 2048]  bf16   c = 128*ci + p
  wq/wk[128, 3, 384]   bf16   packed Wq[h,c,d] -> [c, h*64+d]
  wv/wp[128, 3, 384]   bf16
  biasb[128, 384]      f32    bias replicated across partitions

Per-core compute:
  QT/KT [hd, t] via matmul; Vt [s, h, 65] augmented (col 64 == 1 -> denom row).
  Attention per (j q-block of 512, hp head-pair, i s-chunk of 128):
    S-pair [128, 1024] = two concurrent K=64 matmuls (tile rows 0:64 / 64:128)
    one wide exp (ACT) -> P bf16 [128, 1024]; causal diag via affine_select
    PV per head accumulates O[65, 512] (row 64 = softmax denominator)
  PSUM: 4 banks pure S rotation (2x [128,1024], nothing else touches it until
  the tail), 2 banks O (Oa/Ob), 2 banks aux (round-robin for projection
  drip tasks, recipB broadcasts, and Y accumulations).
  Denominator chain is split + deferred: denom rows -> (next frame i==1)
  reciprocal_approx_fast + fp16 cast -> (i==5) K=1 broadcast matmuls into an
  aux recipB tile -> tensor_mul normalizes attT in place.  Output projection
  is a dense K=128 accumulation Y[t,e] = attT^T @ wp (+bias on eviction),
  scheduled into late chunk slots; the last q-block's Y runs on the freed S
  banks at the tail.  Phase-1 projections follow a per-frame slot plan with
  JIT fallback.
"""

import numpy as np
import ml_dtypes

import concourse.bass as bass
import concourse.tile as tile
from concourse import bacc, mybir
from concourse.bass import ts, ds

F32 = mybir.dt.float32
BF16 = mybir.dt.bfloat16
FP16 = mybir.dt.float16
AF = mybir.ActivationFunctionType

B, T, C = 8, 2048, 384
H, DH = 6, 64
SCALE = DH ** -0.5
NCORES = 8
TJ = 512            # q-block width
NJ = T // TJ        # 4 q-blocks
SC = 128            # s-chunk
NCI = C // 128      # 3 channel chunks
NHP = H // 2        # 3 head pairs (= hd blocks)


def build_kernel():
    nc = bacc.Bacc("TRN2", target_bir_lowering=False, debug=False)

    xT_d = nc.dram_tensor("xT", [128, NCI, T], BF16, kind="ExternalInput").ap()
    wq_d = nc.dram_tensor("wq", [128, NCI, C], BF16, kind="ExternalInput").ap()
    wk_d = nc.dram_tensor("wk", [128, NCI, C], BF16, kind="ExternalInput").ap()
    wv_d = nc.dram_tensor("wv", [128, NCI, C], BF16, kind="ExternalInput").ap()
    wp_d = nc.dram_tensor("wp", [128, NCI, C], BF16, kind="ExternalInput").ap()
    biasb_d = nc.dram_tensor("biasb", [128, C], F32, kind="ExternalInput").ap()
    y_d = nc.dram_tensor("y", [T, C], F32, kind="ExternalOutput").ap()

    with tile.TileContext(nc) as tc:
        with tc.tile_pool(name="const", bufs=1) as cpool, \
             tc.tile_pool(name="pp", bufs=2) as pp, \
             tc.tile_pool(name="sp", bufs=2, space="PSUM") as sp, \
             tc.tile_pool(name="op", bufs=1, space="PSUM") as op, \
             tc.tile_pool(name="ax", bufs=1, space="PSUM") as ax:
            xT = cpool.tile([128, NCI, T], BF16)
            wq = cpool.tile([128, NCI, C], BF16)
            wk = cpool.tile([128, NCI, C], BF16)
            wv = cpool.tile([128, NCI, C], BF16)
            wp = cpool.tile([128, NCI, C], BF16)
            biasb = cpool.tile([128, C], F32)
            QT = cpool.tile([128, NCI, T], BF16)
            KT = cpool.tile([128, NCI, T], BF16)
            attT = cpool.tile([128, NCI, T], BF16)
            Vt = cpool.tile([128, 16, H, 65], BF16)
            ones16 = cpool.tile([1, 64], FP16)
            scr = cpool.tile([1, 64], F32)

            nc.gpsimd.memset(ones16[:], 1.0)
            # preload the exp activation table while DMAs run
            nc.scalar.activation(scr[:], ones16[:], AF.Exp, scale=1.0)
            # whole-tile memset; V copies overwrite cols 0:64 leaving
            # col 64 == 1.0 (denominator trick)
            nc.gpsimd.memset(Vt[:], 1.0)

            nc.sync.dma_start(xT[:, 0, :], xT_d[:, 0, :])
            nc.sync.dma_start(wk[:], wk_d[:])
            nc.sync.dma_start(wq[:], wq_d[:])
            nc.sync.dma_start(xT[:, 1, :], xT_d[:, 1, :])
            nc.sync.dma_start(xT[:, 2, :], xT_d[:, 2, :])
            nc.sync.dma_start(wv[:], wv_d[:])
            nc.sync.dma_start(wp[:], wp_d[:])
            nc.sync.dma_start(biasb[:], biasb_d[:])

            # zero-init both S psum buffers: wide exp calls read full tiles
            # and must never see boot garbage (NaN) even in unused columns
            for _ in range(2):
                z0 = sp.tile([128, 1024], F32, tag="S", name="z0")
                nc.vector.memset(z0[:], 0.0)

            # aux psum: two banks, round-robin
            aux_rr = [0]

            def aux_tile():
                tag = "WA" if aux_rr[0] == 0 else "WB"
                aux_rr[0] ^= 1
                return ax.tile([128, TJ], F32, tag=tag, name="aux")

            # ---- phase-1 projection tasks ----
            done = set()

            def proj_qk(dst, w, bi, q):  # one 512-col quarter
                ps = aux_tile()
                for ci in range(NCI):
                    nc.tensor.matmul(
                        ps[:],
                        lhsT=w[:, ci, ts(bi, 128)],
                        rhs=xT[:, ci, ts(q, 512)],
                        start=(ci == 0), stop=(ci == NCI - 1),
                    )
                nc.vector.tensor_copy(dst[:, bi, ts(q, 512)], ps[:])

            def proj_v(si):
                ps = aux_tile()
                for ci in range(NCI):
                    nc.tensor.matmul(
                        ps[:, 0:C],
                        lhsT=xT[:, ci, ts(si, 128)],
                        rhs=wv[:, ci, :],
                        start=(ci == 0), stop=(ci == NCI - 1),
                    )
                nc.vector.tensor_copy(
                    Vt[:, si, :, 0:64],
                    ps[:, 0:C].rearrange("p (h d) -> p h d", h=H),
                )

            def emit_key(key):
                if key in done:
                    return
                done.add(key)
                if key[0] == 'K':
                    proj_qk(KT, wk, key[1], key[2])
                elif key[0] == 'Q':
                    proj_qk(QT, wq, key[1], key[2])
                else:
                    proj_v(key[1])

            # prefix: exactly what (j=0, hp=0) starts with
            for key in [('K', 0, 0), ('Q', 0, 0), ('V', 0), ('V', 1)]:
                emit_key(key)

            # per-frame drip plan: frame f = 3*j + hp -> ordered task list,
            # consumed one per drip slot (j0: i in 1,2,3; else i in 1,2,4,6,7)
            PLAN = {
                0: [('V', 2), ('K', 1, 0), ('Q', 1, 0)],
                1: [('K', 2, 0), ('Q', 2, 0)],
                2: [('K', 0, 1), ('Q', 0, 1)],
                3: [('V', 4), ('V', 5), ('V', 6), ('V', 7), ('Q', 1, 1)],
                4: [('K', 1, 1), ('K', 2, 1), ('Q', 2, 1), ('V', 8), ('V', 9)],
                5: [('Q', 0, 2), ('K', 0, 2), ('V', 10), ('V', 11)],
                6: [('Q', 1, 2), ('K', 1, 2), ('Q', 0, 3), ('K', 0, 3)],
                7: [('Q', 2, 2), ('K', 2, 2), ('V', 12), ('V', 13)],
                8: [('Q', 1, 3), ('K', 1, 3), ('V', 14), ('V', 15)],
                9: [('Q', 2, 3), ('K', 2, 3)],
            }

            ytasks = []       # deferred output-projection closures
            norm_pre = []     # deferred recip+fp16 cast (DVE only)
            norm_post = []    # deferred broadcast+tensor_mul
            ndone = [0]       # completed normalizations

            def make_y(j_, tb_, tail=False):
                def run():
                    if tail:
                        Yt = sp.tile([128, 1024], F32, tag="S", name="Yt")
                        Y = Yt[:, 0:C]
                    else:
                        Y = aux_tile()[:, 0:C]
                    for bi in range(NCI):
                        nc.tensor.matmul(
                            Y,
                            lhsT=attT[:, bi, ts(4 * j_ + tb_, 128)],
                            rhs=wp[:, bi, :],
                            start=(bi == 0), stop=(bi == NCI - 1),
                        )
                    ysb = pp.tile([128, C], F32, tag="ysb")
                    nc.vector.tensor_add(ysb[:], Y, biasb[:])
                    nc.sync.dma_start(y_d[ts(4 * j_ + tb_, 128), :], ysb[:])
                return run

            # ---- phase 2: attention ----
            for j in range(NJ):
                for hp in range(NHP):
                    f = 3 * j + hp
                    plan = list(PLAN.get(f, []))
                    ha, hb = 2 * hp, 2 * hp + 1
                    # JIT safety net for this frame's K quarters / Q block
                    for q in range(j + 1):
                        emit_key(('K', hp, q))
                    emit_key(('Q', hp, j))
                    Oa = op.tile([65, TJ], F32, tag="Oa")
                    Ob = op.tile([65, TJ], F32, tag="Ob")
                    nch = 4 * j + 4
                    Ps = [None] * nch

                    def emit_S(i):
                        d = max(0, SC * i - TJ * j)
                        st = sp.tile([128, 1024], F32, tag="S")
                        for z in (0, 64):
                            nc.tensor.matmul(
                                st[:, 8 * z + d:8 * z + 512],
                                lhsT=KT[z:z + 64, hp, ts(i, SC)],
                                rhs=QT[z:z + 64, hp, ds(TJ * j + d, TJ - d)],
                                start=True, stop=True,
                            )
                        P = pp.tile([128, 1024], BF16, tag="P")
                        if d <= 128:
                            nc.scalar.activation(P[:], st[:], AF.Exp, scale=SCALE)
                        else:
                            for z in (0, 512):
                                nc.scalar.activation(P[:, z + d:z + 512],
                                                     st[:, z + d:z + 512],
                                                     AF.Exp, scale=SCALE)
                        if SC * i >= TJ * j:  # fringe: mask diag window
                            for z in (0, 512):
                                nc.gpsimd.affine_select(
                                    out=P[:, z + d:z + d + 128],
                                    in_=P[:, z + d:z + d + 128],
                                    pattern=[[1, 128]],
                                    compare_op=mybir.AluOpType.is_ge,
                                    fill=0.0, base=0, channel_multiplier=-1,
                                )
                        Ps[i] = P

                    def emit_PV(i):
                        d = max(0, SC * i - TJ * j)
                        emit_key(('V', i))
                        for O, z, h in ((Oa, 0, ha), (Ob, 512, hb)):
                            nc.tensor.matmul(
                                O[:, d:TJ],
                                lhsT=Vt[:, i, h, :],
                                rhs=Ps[i][:, z + d:z + 512],
                                start=(i == 0), stop=(i == nch - 1),
                            )

                    norm_slot = 3 if j == 0 else 5
                    drip_slots = (1, 2, 3) if j == 0 else (1, 2, 4, 6, 7)
                    for i in range(nch):
                        emit_S(i)
                        if i >= 1:
                            emit_PV(i - 1)
                        if i == 1 and norm_pre:
                            norm_pre.pop(0)()
                        if i in drip_slots and plan:
                            emit_key(plan.pop(0))
                        elif i == norm_slot and norm_post:
                            norm_post.pop(0)()
                        elif i >= 8 and i % 2 == 0 and ytasks and \
                                ndone[0] >= 3 * (ytasks[0][0] + 1):
                            ytasks.pop(0)[1]()
                    emit_PV(nch - 1)

                    # evict attT (unnormalized) + denominator rows (a first:
                    # the next frame's first PV waits on Oa's readers)
                    dsb = pp.tile([1, 2 * TJ], F32, tag="dsb")
                    nc.vector.tensor_copy(attT[0:64, hp, ts(j, TJ)], Oa[0:64, :])
                    nc.vector.tensor_copy(dsb[0:1, 0:TJ], Oa[64:65, :])
                    nc.vector.tensor_copy(attT[64:128, hp, ts(j, TJ)], Ob[0:64, :])
                    nc.vector.tensor_copy(dsb[0:1, TJ:2 * TJ], Ob[64:65, :])

                    def make_pre(dsb_):
                        r16 = pp.tile([1, 2 * TJ], FP16, tag="r16",
                                      name="r16")
                        def run():
                            rr = pp.tile([1, 2 * TJ], F32, tag="rr")
                            nc.vector.reciprocal_approx_fast(rr[:], dsb_[:])
                            nc.vector.tensor_copy(r16[:], rr[:])
                        return run, r16

                    def make_post(j_, hp_, r16_):
                        def run():
                            rB = aux_tile()
                            for z in (0, 64):
                                for q in range(4):
                                    nc.tensor.matmul(
                                        rB[z:z + 64, ts(q, 128)],
                                        lhsT=ones16[0:1, :],
                                        rhs=r16_[0:1, ds(8 * z + 128 * q, 128)],
                                        start=True, stop=True,
                                    )
                            nc.vector.tensor_mul(attT[:, hp_, ts(j_, TJ)],
                                                 attT[:, hp_, ts(j_, TJ)],
                                                 rB[:])
                            ndone[0] += 1
                        return run
                    pre, r16h = make_pre(dsb)
                    norm_pre.append(pre)
                    norm_post.append(make_post(j, hp, r16h))

                for tb in range(4):
                    ytasks.append((j, make_y(j, tb, tail=(j == NJ - 1))))

            while norm_pre:   # tail: last normalize + j=3 output blocks
                norm_pre.pop(0)()
            while norm_post:
                norm_post.pop(0)()
            while ytasks:
                ytasks.pop(0)[1]()

    nc.compile()
    return nc


def _prep_inputs(x, Wq, Wk, Wv, Wp, bp):
    """Host-side shard + layout prep. Returns per-core input maps."""
    bf = ml_dtypes.bfloat16
    x = np.asarray(x, dtype=np.float32)

    def pack_w(W):  # [H, C, Dh] -> [128, NCI, H*Dh]
        Whd = np.transpose(np.asarray(W, np.float32), (1, 0, 2)).reshape(C, H * DH)
        return np.ascontiguousarray(
            Whd.reshape(NCI, 128, H * DH).transpose(1, 0, 2)
        ).astype(bf)

    wq_p, wk_p, wv_p = pack_w(Wq), pack_w(Wk), pack_w(Wv)
    wp_p = np.ascontiguousarray(
        np.asarray(Wp, np.float32).reshape(NCI, 128, C).transpose(1, 0, 2)
    ).astype(bf)
    biasb = np.broadcast_to(np.asarray(bp, np.float32), (128, C)).copy()

    in_maps = []
    for b in range(B):
        xT = np.ascontiguousarray(
            x[b].T.reshape(NCI, 128, T).transpose(1, 0, 2)
        ).astype(bf)
        in_maps.append({
            "xT": xT, "wq": wq_p, "wk": wk_p, "wv": wv_p, "wp": wp_p,
            "biasb": biasb,
        })
    return in_maps


_CACHE = {}


def kernel(x, Wq, Wk, Wv, Wp, bp):
    from concourse.bass_utils import run_bass_kernel_spmd

    if "nc" not in _CACHE:
        _CACHE["nc"] = build_kernel()
    nc = _CACHE["nc"]
    in_maps = _prep_inputs(x, Wq, Wk, Wv, Wp, bp)
    res = run_bass_kernel_spmd(nc, in_maps, list(range(NCORES)))
    out = np.stack([res.results[b]["y"] for b in range(B)], axis=0)
    return out.astype(np.float32)


# revision 11
# speedup vs baseline: 1.1628x; 1.0124x over previous
"""Multi-head causal attention (B=8, T=2048, C=384, H=6, Dh=64) on 8 TRN2 cores.

Sharding: data-parallel over batch - core b computes batch element b end to end
(no collectives).

v5 layout (all "T" means transposed, head-dim/channel on partitions):
  xT   [128, 3, 2048]  bf16   c = 128*ci + p
  wq/wk[128, 3, 384]   bf16   packed Wq[h,c,d] -> [c, h*64+d]
  wv/wp[128, 3, 384]   bf16
  biasb[128, 384]      f32    bias replicated across partitions

Per-core compute:
  QT/KT [hd, t] via matmul; Vt [s, h, 65] augmented (col 64 == 1 -> denom row).
  Attention per (j q-block of 512, hp head-pair, i s-chunk of 128):
    S-pair [128, 1024] = two concurrent K=64 matmuls (tile rows 0:64 / 64:128)
    one wide exp (ACT) -> P bf16 [128, 1024]; causal diag via affine_select
    PV per head accumulates O[65, 512] (row 64 = softmax denominator)
  PSUM: 4 banks pure S rotation (2x [128,1024], nothing else touches it until
  the tail), 2 banks O (Oa/Ob), 2 banks aux (round-robin for projection
  drip tasks, recipB broadcasts, and Y accumulations).
  Denominator chain is split + deferred: denom rows -> (next frame i==1)
  reciprocal_approx_fast + fp16 cast -> (i==5) K=1 broadcast matmuls into an
  aux recipB tile -> tensor_mul normalizes attT in place.  Output projection
  is a dense K=128 accumulation Y[t,e] = attT^T @ wp (+bias on eviction),
  scheduled into late chunk slots; the last q-block's Y runs on the freed S
  banks at the tail.  Phase-1 projections follow a per-frame slot plan with
  JIT fallback.
"""

import numpy as np
import ml_dtypes

import concourse.bass as bass
import concourse.tile as tile
from concourse import bacc, mybir
from concourse.bass import ts, ds

F32 = mybir.dt.float32
BF16 = mybir.dt.bfloat16
FP16 = mybir.dt.float16
AF = mybir.ActivationFunctionType

B, T, C = 8, 2048, 384
H, DH = 6, 64
SCALE = DH ** -0.5
NCORES = 8
TJ = 512            # q-block width
NJ = T // TJ        # 4 q-blocks
SC = 128            # s-chunk
NCI = C // 128      # 3 channel chunks
NHP = H // 2        # 3 head pairs (= hd blocks)


def build_kernel():
    nc = bacc.Bacc("TRN2", target_bir_lowering=False, debug=False)

    xT_d = nc.dram_tensor("xT", [128, NCI, T], BF16, kind="ExternalInput").ap()
    wq_d = nc.dram_tensor("wq", [128, NCI, C], BF16, kind="ExternalInput").ap()
    wk_d = nc.dram_tensor("wk", [128, NCI, C], BF16, kind="ExternalInput").ap()
    wv_d = nc.dram_tensor("wv", [128, NCI, C], BF16, kind="ExternalInput").ap()
    wp_d = nc.dram_tensor("wp", [128, NCI, C], BF16, kind="ExternalInput").ap()
    biasb_d = nc.dram_tensor("biasb", [128, C], F32, kind="ExternalInput").ap()
    y_d = nc.dram_tensor("y", [T, C], F32, kind="ExternalOutput").ap()

    with tile.TileContext(nc) as tc:
        with tc.tile_pool(name="const", bufs=1) as cpool, \
             tc.tile_pool(name="pp", bufs=2) as pp, \
             tc.tile_pool(name="sp", bufs=2, space="PSUM") as sp, \
             tc.tile_pool(name="op", bufs=1, space="PSUM") as op, \
             tc.tile_pool(name="ax", bufs=1, space="PSUM") as ax:
            xT = cpool.tile([128, NCI, T], BF16)
            wq = cpool.tile([128, NCI, C], BF16)
            wk = cpool.tile([128, NCI, C], BF16)
            wv = cpool.tile([128, NCI, C], BF16)
            wp = cpool.tile([128, NCI, C], BF16)
            biasb = cpool.tile([128, C], F32)
            QT = cpool.tile([128, NCI, T], BF16)
            KT = cpool.tile([128, NCI, T], BF16)
            attT = cpool.tile([128, NCI, T], BF16)
            Vt = cpool.tile([128, 16, H, 65], BF16)
            ones16 = cpool.tile([1, 64], FP16)
            scr = cpool.tile([1, 64], F32)

            nc.gpsimd.memset(ones16[:], 1.0)
            # preload the exp activation table while DMAs run
            nc.scalar.activation(scr[:], ones16[:], AF.Exp, scale=1.0)
            # whole-tile memset; V copies overwrite cols 0:64 leaving
            # col 64 == 1.0 (denominator trick)
            nc.gpsimd.memset(Vt[:], 1.0)

            # halves of xT so the projection prefix starts on partial data
            nc.sync.dma_start(xT[:, 0, 0:1024], xT_d[:, 0, 0:1024])
            nc.sync.dma_start(wk[:], wk_d[:])
            nc.sync.dma_start(wq[:], wq_d[:])
            nc.sync.dma_start(xT[:, 1, 0:1024], xT_d[:, 1, 0:1024])
            nc.sync.dma_start(xT[:, 2, 0:1024], xT_d[:, 2, 0:1024])
            nc.sync.dma_start(wv[:], wv_d[:])
            nc.sync.dma_start(xT[:, 0, 1024:T], xT_d[:, 0, 1024:T])
            nc.sync.dma_start(xT[:, 1, 1024:T], xT_d[:, 1, 1024:T])
            nc.sync.dma_start(xT[:, 2, 1024:T], xT_d[:, 2, 1024:T])
            nc.sync.dma_start(wp[:], wp_d[:])
            nc.sync.dma_start(biasb[:], biasb_d[:])

            # zero-init both S psum buffers: wide exp calls read full tiles
            # and must never see boot garbage (NaN) even in unused columns
            for _ in range(2):
                z0 = sp.tile([128, 1024], F32, tag="S", name="z0")
                nc.vector.memset(z0[:], 0.0)

            # aux psum: two banks, round-robin
            aux_rr = [0]

            def aux_tile():
                tag = "WA" if aux_rr[0] == 0 else "WB"
                aux_rr[0] ^= 1
                return ax.tile([128, TJ], F32, tag=tag, name="aux")

            # ---- phase-1 projection tasks ----
            done = set()

            def proj_qk(dst, w, bi, q):  # one 512-col quarter
                ps = aux_tile()
                for ci in range(NCI):
                    nc.tensor.matmul(
                        ps[:],
                        lhsT=w[:, ci, ts(bi, 128)],
                        rhs=xT[:, ci, ts(q, 512)],
                        start=(ci == 0), stop=(ci == NCI - 1),
                    )
                nc.vector.tensor_copy(dst[:, bi, ts(q, 512)], ps[:])

            def proj_v(si):
                ps = aux_tile()
                for ci in range(NCI):
                    nc.tensor.matmul(
                        ps[:, 0:C],
                        lhsT=xT[:, ci, ts(si, 128)],
                        rhs=wv[:, ci, :],
                        start=(ci == 0), stop=(ci == NCI - 1),
                    )
                nc.vector.tensor_copy(
                    Vt[:, si, :, 0:64],
                    ps[:, 0:C].rearrange("p (h d) -> p h d", h=H),
                )

            def emit_key(key):
                if key in done:
                    return
                done.add(key)
                if key[0] == 'K':
                    proj_qk(KT, wk, key[1], key[2])
                elif key[0] == 'Q':
                    proj_qk(QT, wq, key[1], key[2])
                else:
                    proj_v(key[1])

            # prefix: exactly what (j=0, hp=0) starts with
            for key in [('K', 0, 0), ('Q', 0, 0), ('V', 0), ('V', 1)]:
                emit_key(key)

            # per-frame drip plan: frame f = 3*j + hp -> ordered task list,
            # consumed one per drip slot (j0: i in 1,2,3; else i in 1,2,4,6,7)
            PLAN = {
                0: [('V', 2), ('K', 1, 0), ('Q', 1, 0)],
                1: [('K', 2, 0), ('Q', 2, 0)],
                2: [('K', 0, 1), ('Q', 0, 1)],
                3: [('V', 4), ('V', 5), ('V', 6), ('V', 7), ('Q', 1, 1)],
                4: [('K', 1, 1), ('K', 2, 1), ('Q', 2, 1), ('V', 8), ('V', 9)],
                5: [('Q', 0, 2), ('K', 0, 2), ('V', 10), ('V', 11)],
                6: [('Q', 1, 2), ('K', 1, 2), ('Q', 0, 3), ('K', 0, 3)],
                7: [('Q', 2, 2), ('K', 2, 2), ('V', 12), ('V', 13)],
                8: [('Q', 1, 3), ('K', 1, 3), ('V', 14), ('V', 15)],
                9: [('Q', 2, 3), ('K', 2, 3)],
            }

            ytasks = []       # deferred output-projection closures
            norm_pre = []     # deferred recip+fp16 cast (DVE only)
            norm_post = []    # deferred broadcast+tensor_mul
            ndone = [0]       # completed normalizations

            def make_y(j_, tb_, tail=False):
                def run():
                    if tail and tb_ >= 2:  # spread tail Y over all free banks
                        Yt = sp.tile([128, 1024], F32, tag="S", name="Yt")
                        Y = Yt[:, 0:C]
                    else:
                        Y = aux_tile()[:, 0:C]
                    for bi in range(NCI):
                        nc.tensor.matmul(
                            Y,
                            lhsT=attT[:, bi, ts(4 * j_ + tb_, 128)],
                            rhs=wp[:, bi, :],
                            start=(bi == 0), stop=(bi == NCI - 1),
                        )
                    ysb = pp.tile([128, C], F32, tag="ysb")
                    nc.vector.tensor_add(ysb[:], Y, biasb[:])
                    nc.sync.dma_start(y_d[ts(4 * j_ + tb_, 128), :], ysb[:])
                return run

            # ---- phase 2: attention ----
            for j in range(NJ):
                for hp in range(NHP):
                    f = 3 * j + hp
                    plan = list(PLAN.get(f, []))
                    ha, hb = 2 * hp, 2 * hp + 1
                    # JIT safety net for this frame's K quarters / Q block
                    for q in range(j + 1):
                        emit_key(('K', hp, q))
                    emit_key(('Q', hp, j))
                    Oa = op.tile([65, TJ], F32, tag="Oa")
                    Ob = op.tile([65, TJ], F32, tag="Ob")
                    nch = 4 * j + 4
                    Ps = [None] * nch

                    def emit_S(i):
                        d = max(0, SC * i - TJ * j)
                        st = sp.tile([128, 1024], F32, tag="S")
                        for z in (0, 64):
                            nc.tensor.matmul(
                                st[:, 8 * z + d:8 * z + 512],
                                lhsT=KT[z:z + 64, hp, ts(i, SC)],
                                rhs=QT[z:z + 64, hp, ds(TJ * j + d, TJ - d)],
                                start=True, stop=True,
                            )
                        P = pp.tile([128, 1024], BF16, tag="P")
                        if d <= 128:
                            nc.scalar.activation(P[:], st[:], AF.Exp, scale=SCALE)
                        else:
                            for z in (0, 512):
                                nc.scalar.activation(P[:, z + d:z + 512],
                                                     st[:, z + d:z + 512],
                                                     AF.Exp, scale=SCALE)
                        if SC * i >= TJ * j:  # fringe: mask diag window
                            for z in (0, 512):
                                nc.gpsimd.affine_select(
                                    out=P[:, z + d:z + d + 128],
                                    in_=P[:, z + d:z + d + 128],
                                    pattern=[[1, 128]],
                                    compare_op=mybir.AluOpType.is_ge,
                                    fill=0.0, base=0, channel_multiplier=-1,
                                )
                        Ps[i] = P

                    def emit_PV(i):
                        d = max(0, SC * i - TJ * j)
                        emit_key(('V', i))
                        for O, z, h in ((Oa, 0, ha), (Ob, 512, hb)):
                            nc.tensor.matmul(
                                O[:, d:TJ],
                                lhsT=Vt[:, i, h, :],
                                rhs=Ps[i][:, z + d:z + 512],
                                start=(i == 0), stop=(i == nch - 1),
                            )

                    norm_slot = 3 if j == 0 else 5
                    drip_slots = (1, 2, 3) if j == 0 else (1, 2, 4, 6, 7)
                    for i in range(nch):
                        emit_S(i)
                        if i >= 1:
                            emit_PV(i - 1)
                        if i == 1 and norm_pre:
                            norm_pre.pop(0)()
                        if i in drip_slots and plan:
                            emit_key(plan.pop(0))
                        elif i == norm_slot and norm_post:
                            norm_post.pop(0)()
                        elif i >= 8 and i % 2 == 0 and ytasks and \
                                ndone[0] >= 3 * (ytasks[0][0] + 1):
                            ytasks.pop(0)[1]()
                    emit_PV(nch - 1)

                    # evict attT (unnormalized) + denominator rows (a first:
                    # the next frame's first PV waits on Oa's readers)
                    dsb = pp.tile([1, 2 * TJ], F32, tag="dsb")
                    nc.vector.tensor_copy(attT[0:64, hp, ts(j, TJ)], Oa[0:64, :])
                    nc.vector.tensor_copy(dsb[0:1, 0:TJ], Oa[64:65, :])
                    nc.vector.tensor_copy(attT[64:128, hp, ts(j, TJ)], Ob[0:64, :])
                    nc.vector.tensor_copy(dsb[0:1, TJ:2 * TJ], Ob[64:65, :])

                    def make_pre(dsb_):
                        r16 = pp.tile([1, 2 * TJ], FP16, tag="r16",
                                      name="r16")
                        def run():
                            rr = pp.tile([1, 2 * TJ], F32, tag="rr")
                            nc.vector.reciprocal_approx_fast(rr[:], dsb_[:])
                            nc.vector.tensor_copy(r16[:], rr[:])
                        return run, r16

                    def make_post(j_, hp_, r16_):
                        def run():
                            rB = aux_tile()
                            for z in (0, 64):
                                for q in range(4):
                                    nc.tensor.matmul(
                                        rB[z:z + 64, ts(q, 128)],
                                        lhsT=ones16[0:1, :],
                                        rhs=r16_[0:1, ds(8 * z + 128 * q, 128)],
                                        start=True, stop=True,
                                    )
                            nc.vector.tensor_mul(attT[:, hp_, ts(j_, TJ)],
                                                 attT[:, hp_, ts(j_, TJ)],
                                                 rB[:])
                            ndone[0] += 1
                        return run
                    pre, r16h = make_pre(dsb)
                    norm_pre.append(pre)
                    norm_post.append(make_post(j, hp, r16h))

                for tb in range(4):
                    ytasks.append((j, make_y(j, tb, tail=(j == NJ - 1))))

            while norm_pre:   # tail: last normalize + j=3 output blocks
                norm_pre.pop(0)()
            while norm_post:
                norm_post.pop(0)()
            while ytasks:
                ytasks.pop(0)[1]()

    nc.compile()
    return nc


def _prep_inputs(x, Wq, Wk, Wv, Wp, bp):
    """Host-side shard + layout prep. Returns per-core input maps."""
    bf = ml_dtypes.bfloat16
    x = np.asarray(x, dtype=np.float32)

    def pack_w(W):  # [H, C, Dh] -> [128, NCI, H*Dh]
        Whd = np.transpose(np.asarray(W, np.float32), (1, 0, 2)).reshape(C, H * DH)
        return np.ascontiguousarray(
            Whd.reshape(NCI, 128, H * DH).transpose(1, 0, 2)
        ).astype(bf)

    wq_p, wk_p, wv_p = pack_w(Wq), pack_w(Wk), pack_w(Wv)
    wp_p = np.ascontiguousarray(
        np.asarray(Wp, np.float32).reshape(NCI, 128, C).transpose(1, 0, 2)
    ).astype(bf)
    biasb = np.broadcast_to(np.asarray(bp, np.float32), (128, C)).copy()

    in_maps = []
    for b in range(B):
        xT = np.ascontiguousarray(
            x[b].T.reshape(NCI, 128, T).transpose(1, 0, 2)
        ).astype(bf)
        in_maps.append({
            "xT": xT, "wq": wq_p, "wk": wk_p, "wv": wv_p, "wp": wp_p,
            "biasb": biasb,
        })
    return in_maps


_CACHE = {}


def kernel(x, Wq, Wk, Wv, Wp, bp):
    from concourse.bass_utils import run_bass_kernel_spmd

    if "nc" not in _CACHE:
        _CACHE["nc"] = build_kernel()
    nc = _CACHE["nc"]
    in_maps = _prep_inputs(x, Wq, Wk, Wv, Wp, bp)
    res = run_bass_kernel_spmd(nc, in_maps, list(range(NCORES)))
    out = np.stack([res.results[b]["y"] for b in range(B)], axis=0)
    return out.astype(np.float32)


# revision 13
# speedup vs baseline: 1.1766x; 1.0118x over previous
"""Multi-head causal attention (B=8, T=2048, C=384, H=6, Dh=64) on 8 TRN2 cores.

Sharding: data-parallel over batch - core b computes batch element b end to end
(no collectives).

v5 layout (all "T" means transposed, head-dim/channel on partitions):
  xT   [128, 3, 2048]  bf16   c = 128*ci + p
  wq/wk[128, 3, 384]   bf16   packed Wq[h,c,d] -> [c, h*64+d]
  wv/wp[128, 3, 384]   bf16
  biasb[128, 384]      f32    bias replicated across partitions

Per-core compute:
  QT/KT [hd, t] via matmul; Vt [s, h, 65] augmented (col 64 == 1 -> denom row).
  Attention per (j q-block of 512, hp head-pair, i s-chunk of 128):
    S-pair [128, 1024] = two concurrent K=64 matmuls (tile rows 0:64 / 64:128)
    one wide exp (ACT) -> P bf16 [128, 1024]; causal diag via affine_select
    PV per head accumulates O[65, 512] (row 64 = softmax denominator)
  PSUM: 4 banks pure S rotation (2x [128,1024], nothing else touches it until
  the tail), 2 banks O (Oa/Ob), 2 banks aux (round-robin for projection
  drip tasks, recipB broadcasts, and Y accumulations).
  Denominator chain is split + deferred: denom rows -> (next frame i==1)
  reciprocal_approx_fast + fp16 cast -> (i==5) K=1 broadcast matmuls into an
  aux recipB tile -> tensor_mul normalizes attT in place.  Output projection
  is a dense K=128 accumulation Y[t,e] = attT^T @ wp (+bias on eviction),
  scheduled into late chunk slots; the last q-block's Y runs on the freed S
  banks at the tail.  Phase-1 projections follow a per-frame slot plan with
  JIT fallback.
"""

import numpy as np
import ml_dtypes

import concourse.bass as bass
import concourse.tile as tile
from concourse import bacc, mybir
from concourse.bass import ts, ds

F32 = mybir.dt.float32
BF16 = mybir.dt.bfloat16
FP16 = mybir.dt.float16
AF = mybir.ActivationFunctionType

B, T, C = 8, 2048, 384
H, DH = 6, 64
SCALE = DH ** -0.5
NCORES = 8
TJ = 512            # q-block width
NJ = T // TJ        # 4 q-blocks
SC = 128            # s-chunk
NCI = C // 128      # 3 channel chunks
NHP = H // 2        # 3 head pairs (= hd blocks)


def build_kernel():
    nc = bacc.Bacc("TRN2", target_bir_lowering=False, debug=False)

    xT_d = nc.dram_tensor("xT", [128, NCI, T], BF16, kind="ExternalInput").ap()
    wq_d = nc.dram_tensor("wq", [128, NCI, C], BF16, kind="ExternalInput").ap()
    wk_d = nc.dram_tensor("wk", [128, NCI, C], BF16, kind="ExternalInput").ap()
    wv_d = nc.dram_tensor("wv", [128, NCI, C], BF16, kind="ExternalInput").ap()
    wp_d = nc.dram_tensor("wp", [128, NCI, C], BF16, kind="ExternalInput").ap()
    biasb_d = nc.dram_tensor("biasb", [128, C], F32, kind="ExternalInput").ap()
    y_d = nc.dram_tensor("y", [T, C], F32, kind="ExternalOutput").ap()

    with tile.TileContext(nc) as tc:
        with tc.tile_pool(name="const", bufs=1) as cpool, \
             tc.tile_pool(name="pp", bufs=2) as pp, \
             tc.tile_pool(name="sp", bufs=2, space="PSUM") as sp, \
             tc.tile_pool(name="op", bufs=1, space="PSUM") as op, \
             tc.tile_pool(name="ax", bufs=1, space="PSUM") as ax:
            xT = cpool.tile([128, NCI, T], BF16)
            wq = cpool.tile([128, NCI, C], BF16)
            wk = cpool.tile([128, NCI, C], BF16)
            wv = cpool.tile([128, NCI, C], BF16)
            wp = cpool.tile([128, NCI, C], BF16)
            biasb = cpool.tile([128, C], F32)
            QT = cpool.tile([128, NCI, T], BF16)
            KT = cpool.tile([128, NCI, T], BF16)
            attT = cpool.tile([128, NCI, T], BF16)
            Vt = cpool.tile([128, 16, H, 65], BF16)
            ones16 = cpool.tile([1, 64], FP16)
            scr = cpool.tile([1, 64], F32)

            nc.gpsimd.memset(ones16[:], 1.0)
            # preload the exp activation table while DMAs run
            nc.scalar.activation(scr[:], ones16[:], AF.Exp, scale=1.0)
            # whole-tile memset; V copies overwrite cols 0:64 leaving
            # col 64 == 1.0 (denominator trick)
            nc.gpsimd.memset(Vt[:], 1.0)

            # halves of xT so the projection prefix starts on partial data
            nc.sync.dma_start(xT[:, 0, 0:1024], xT_d[:, 0, 0:1024])
            nc.sync.dma_start(wk[:], wk_d[:])
            nc.sync.dma_start(wq[:], wq_d[:])
            nc.sync.dma_start(xT[:, 1, 0:1024], xT_d[:, 1, 0:1024])
            nc.sync.dma_start(xT[:, 2, 0:1024], xT_d[:, 2, 0:1024])
            nc.sync.dma_start(wv[:], wv_d[:])
            nc.sync.dma_start(xT[:, 0, 1024:T], xT_d[:, 0, 1024:T])
            nc.sync.dma_start(xT[:, 1, 1024:T], xT_d[:, 1, 1024:T])
            nc.sync.dma_start(xT[:, 2, 1024:T], xT_d[:, 2, 1024:T])
            nc.sync.dma_start(wp[:], wp_d[:])
            nc.sync.dma_start(biasb[:], biasb_d[:])

            # zero-init both S psum buffers: wide exp calls read full tiles
            # and must never see boot garbage (NaN) even in unused columns
            for _ in range(2):
                z0 = sp.tile([128, 1024], F32, tag="S", name="z0")
                nc.vector.memset(z0[:], 0.0)

            # aux psum: two banks, round-robin
            aux_rr = [0]

            def aux_tile():
                tag = "WA" if aux_rr[0] == 0 else "WB"
                aux_rr[0] ^= 1
                return ax.tile([128, TJ], F32, tag=tag, name="aux")

            # ---- phase-1 projection tasks ----
            done = set()

            def proj_qk(dst, w, bi, q):  # one 512-col quarter
                ps = aux_tile()
                for ci in range(NCI):
                    nc.tensor.matmul(
                        ps[:],
                        lhsT=w[:, ci, ts(bi, 128)],
                        rhs=xT[:, ci, ts(q, 512)],
                        start=(ci == 0), stop=(ci == NCI - 1),
                    )
                nc.vector.tensor_copy(dst[:, bi, ts(q, 512)], ps[:])

            def proj_v(si):
                ps = aux_tile()
                for ci in range(NCI):
                    nc.tensor.matmul(
                        ps[:, 0:C],
                        lhsT=xT[:, ci, ts(si, 128)],
                        rhs=wv[:, ci, :],
                        start=(ci == 0), stop=(ci == NCI - 1),
                    )
                nc.vector.tensor_copy(
                    Vt[:, si, :, 0:64],
                    ps[:, 0:C].rearrange("p (h d) -> p h d", h=H),
                )

            def emit_key(key):
                if key in done:
                    return
                done.add(key)
                if key[0] == 'K':
                    proj_qk(KT, wk, key[1], key[2])
                elif key[0] == 'Q':
                    proj_qk(QT, wq, key[1], key[2])
                else:
                    proj_v(key[1])

            # prefix: exactly what (j=0, hp=0) starts with
            for key in [('K', 0, 0), ('Q', 0, 0), ('V', 0), ('V', 1)]:
                emit_key(key)

            # per-frame drip plan: frame f = 3*j + hp -> ordered task list,
            # consumed one per drip slot (j0: i in 1,2,3; else i in 1,2,4,6,7)
            PLAN = {
                0: [('V', 2), ('K', 1, 0), ('Q', 1, 0)],
                1: [('K', 2, 0), ('Q', 2, 0)],
                2: [('K', 0, 1), ('Q', 0, 1)],
                3: [('V', 4), ('V', 5), ('V', 6), ('V', 7), ('Q', 1, 1)],
                4: [('K', 1, 1), ('K', 2, 1), ('Q', 2, 1), ('V', 8), ('V', 9)],
                5: [('Q', 0, 2), ('K', 0, 2), ('V', 10), ('V', 11)],
                6: [('Q', 1, 2), ('K', 1, 2), ('Q', 0, 3), ('K', 0, 3)],
                7: [('Q', 2, 2), ('K', 2, 2), ('V', 12), ('V', 13)],
                8: [('Q', 1, 3), ('K', 1, 3), ('V', 14), ('V', 15)],
                9: [('Q', 2, 3), ('K', 2, 3)],
            }

            ytasks = []       # deferred output-projection closures
            norm_pre = []     # deferred recip+fp16 cast (DVE only)
            norm_post = []    # deferred broadcast+tensor_mul
            ndone = [0]       # completed normalizations

            def make_y(j_, tb_, tail=False):
                def run():
                    if tail and tb_ >= 2:  # spread tail Y over all free banks
                        Yt = sp.tile([128, 1024], F32, tag="S", name="Yt")
                        Y = Yt[:, 0:C]
                    else:
                        Y = aux_tile()[:, 0:C]
                    for bi in range(NCI):
                        nc.tensor.matmul(
                            Y,
                            lhsT=attT[:, bi, ts(4 * j_ + tb_, 128)],
                            rhs=wp[:, bi, :],
                            start=(bi == 0), stop=(bi == NCI - 1),
                        )
                    ysb = pp.tile([128, C], F32, tag="ysb")
                    nc.vector.tensor_add(ysb[:], Y, biasb[:])
                    nc.sync.dma_start(y_d[ts(4 * j_ + tb_, 128), :], ysb[:])
                return run

            # ---- phase 2: attention, flat chunk stream with cross-frame
            # S-priming (each chunk's S-pair emitted one iteration ahead, so
            # the next frame's exps never wait on the previous frame's last
            # PV or eviction chain) ----
            chunks = [(j, hp, i, 4 * j + 4)
                      for j in range(NJ) for hp in range(NHP)
                      for i in range(4 * j + 4)]
            Ps = {}
            frame = {}  # current PV-side frame state

            def emit_S(j, hp, i, nch):
                if i == 0:  # JIT safety net for the new frame's K/Q
                    for q in range(j + 1):
                        emit_key(('K', hp, q))
                    emit_key(('Q', hp, j))
                d = max(0, SC * i - TJ * j)
                st = sp.tile([128, 1024], F32, tag="S")
                for z in (0, 64):
                    nc.tensor.matmul(
                        st[:, 8 * z + d:8 * z + 512],
                        lhsT=KT[z:z + 64, hp, ts(i, SC)],
                        rhs=QT[z:z + 64, hp, ds(TJ * j + d, TJ - d)],
                        start=True, stop=True,
                    )
                P = pp.tile([128, 1024], BF16, tag="P")
                if d <= 128:
                    nc.scalar.activation(P[:], st[:], AF.Exp, scale=SCALE)
                else:
                    for z in (0, 512):
                        nc.scalar.activation(P[:, z + d:z + 512],
                                             st[:, z + d:z + 512],
                                             AF.Exp, scale=SCALE)
                if SC * i >= TJ * j:  # fringe: mask diag window
                    for z in (0, 512):
                        nc.gpsimd.affine_select(
                            out=P[:, z + d:z + d + 128],
                            in_=P[:, z + d:z + d + 128],
                            pattern=[[1, 128]],
                            compare_op=mybir.AluOpType.is_ge,
                            fill=0.0, base=0, channel_multiplier=-1,
                        )
                Ps[(j, hp, i)] = P

            def emit_PV(j, hp, i, nch):
                if i == 0:
                    frame['Oa'] = op.tile([65, TJ], F32, tag="Oa", name="Oa")
                    frame['Ob'] = op.tile([65, TJ], F32, tag="Ob", name="Ob")
                d = max(0, SC * i - TJ * j)
                emit_key(('V', i))
                P = Ps.pop((j, hp, i))
                for O, z, h in ((frame['Oa'], 0, 2 * hp),
                                (frame['Ob'], 512, 2 * hp + 1)):
                    nc.tensor.matmul(
                        O[:, d:TJ],
                        lhsT=Vt[:, i, h, :],
                        rhs=P[:, z + d:z + 512],
                        start=(i == 0), stop=(i == nch - 1),
                    )

            def make_pre(dsb_):
                r16 = pp.tile([1, 2 * TJ], FP16, tag="r16", name="r16")
                def run():
                    rr = pp.tile([1, 2 * TJ], F32, tag="rr")
                    nc.vector.reciprocal_approx_fast(rr[:], dsb_[:])
                    nc.vector.tensor_copy(r16[:], rr[:])
                return run, r16

            def make_post(j_, hp_, r16_):
                def run():
                    rB = aux_tile()
                    for z in (0, 64):
                        for q in range(4):
                            nc.tensor.matmul(
                                rB[z:z + 64, ts(q, 128)],
                                lhsT=ones16[0:1, :],
                                rhs=r16_[0:1, ds(8 * z + 128 * q, 128)],
                                start=True, stop=True,
                            )
                    nc.vector.tensor_mul(attT[:, hp_, ts(j_, TJ)],
                                         attT[:, hp_, ts(j_, TJ)],
                                         rB[:])
                    ndone[0] += 1
                return run

            plan = []
            emit_S(*chunks[0])
            for k, (j, hp, i, nch) in enumerate(chunks):
                if k + 1 < len(chunks):
                    emit_S(*chunks[k + 1])
                emit_PV(j, hp, i, nch)
                if i == 0:
                    plan = list(PLAN.get(3 * j + hp, []))
                norm_slot = 3 if j == 0 else 5
                drip_slots = (1, 2, 3) if j == 0 else (1, 2, 4, 6, 7)
                if i == 1 and norm_pre:
                    norm_pre.pop(0)()
                if i in drip_slots and plan:
                    emit_key(plan.pop(0))
                elif i == norm_slot and norm_post:
                    norm_post.pop(0)()
                elif i >= 8 and i % 2 == 0 and ytasks and \
                        ndone[0] >= 3 * (ytasks[0][0] + 1):
                    ytasks.pop(0)[1]()
                if i == nch - 1:
                    # evict attT (unnormalized) + denominator rows (a first:
                    # the next frame's first PV waits on Oa's readers)
                    Oa, Ob = frame['Oa'], frame['Ob']
                    dsb = pp.tile([1, 2 * TJ], F32, tag="dsb")
                    nc.vector.tensor_copy(attT[0:64, hp, ts(j, TJ)],
                                          Oa[0:64, :])
                    nc.vector.tensor_copy(dsb[0:1, 0:TJ], Oa[64:65, :])
                    nc.vector.tensor_copy(attT[64:128, hp, ts(j, TJ)],
                                          Ob[0:64, :])
                    nc.vector.tensor_copy(dsb[0:1, TJ:2 * TJ], Ob[64:65, :])
                    pre, r16h = make_pre(dsb)
                    norm_pre.append(pre)
                    norm_post.append(make_post(j, hp, r16h))
                    if hp == NHP - 1:
                        for tb in range(4):
                            ytasks.append(
                                (j, make_y(j, tb, tail=(j == NJ - 1))))

            while norm_pre:   # tail: last normalize + j=3 output blocks
                norm_pre.pop(0)()
            while norm_post:
                norm_post.pop(0)()
            while ytasks:
                ytasks.pop(0)[1]()

    nc.compile()
    return nc


def _prep_inputs(x, Wq, Wk, Wv, Wp, bp):
    """Host-side shard + layout prep. Returns per-core input maps."""
    bf = ml_dtypes.bfloat16
    x = np.asarray(x, dtype=np.float32)

    def pack_w(W):  # [H, C, Dh] -> [128, NCI, H*Dh]
        Whd = np.transpose(np.asarray(W, np.float32), (1, 0, 2)).reshape(C, H * DH)
        return np.ascontiguousarray(
            Whd.reshape(NCI, 128, H * DH).transpose(1, 0, 2)
        ).astype(bf)

    wq_p, wk_p, wv_p = pack_w(Wq), pack_w(Wk), pack_w(Wv)
    wp_p = np.ascontiguousarray(
        np.asarray(Wp, np.float32).reshape(NCI, 128, C).transpose(1, 0, 2)
    ).astype(bf)
    biasb = np.broadcast_to(np.asarray(bp, np.float32), (128, C)).copy()

    in_maps = []
    for b in range(B):
        xT = np.ascontiguousarray(
            x[b].T.reshape(NCI, 128, T).transpose(1, 0, 2)
        ).astype(bf)
        in_maps.append({
            "xT": xT, "wq": wq_p, "wk": wk_p, "wv": wv_p, "wp": wp_p,
            "biasb": biasb,
        })
    return in_maps


_CACHE = {}


def kernel(x, Wq, Wk, Wv, Wp, bp):
    from concourse.bass_utils import run_bass_kernel_spmd

    if "nc" not in _CACHE:
        _CACHE["nc"] = build_kernel()
    nc = _CACHE["nc"]
    in_maps = _prep_inputs(x, Wq, Wk, Wv, Wp, bp)
    res = run_bass_kernel_spmd(nc, in_maps, list(range(NCORES)))
    out = np.stack([res.results[b]["y"] for b in range(B)], axis=0)
    return out.astype(np.float32)


# revision 15
# speedup vs baseline: 1.2950x; 1.1006x over previous
"""Multi-head causal attention (B=8, T=2048, C=384, H=6, Dh=64) on 8 TRN2 cores.

Sharding: data-parallel over batch - core b computes batch element b end to end
(no collectives).

v5 layout (all "T" means transposed, head-dim/channel on partitions):
  xT   [128, 3, 2048]  bf16   c = 128*ci + p
  wq/wk[128, 3, 384]   bf16   packed Wq[h,c,d] -> [c, h*64+d]
  wv/wp[128, 3, 384]   bf16
  biasb[128, 384]      f32    bias replicated across partitions

Per-core compute:
  QT/KT [hd, t] via matmul; Vt [s, h, 65] augmented (col 64 == 1 -> denom row).
  Attention per (j q-block of 512, hp head-pair, i s-chunk of 128):
    S-pair [128, 1024] = two concurrent K=64 matmuls (tile rows 0:64 / 64:128)
    one wide exp (ACT) -> P bf16 [128, 1024]; causal diag via affine_select
    PV per head accumulates O[65, 512] (row 64 = softmax denominator)
  PSUM: 4 banks pure S rotation (2x [128,1024], nothing else touches it until
  the tail), 2 banks O (Oa/Ob), 2 banks aux (round-robin for projection
  drip tasks, recipB broadcasts, and Y accumulations).
  Denominator chain is split + deferred: denom rows -> (next frame i==1)
  reciprocal_approx_fast + fp16 cast -> (i==5) K=1 broadcast matmuls into an
  aux recipB tile -> tensor_mul normalizes attT in place.  Output projection
  is a dense K=128 accumulation Y[t,e] = attT^T @ wp (+bias on eviction),
  scheduled into late chunk slots; the last q-block's Y runs on the freed S
  banks at the tail.  Phase-1 projections follow a per-frame slot plan with
  JIT fallback.
"""

import numpy as np
import ml_dtypes

import concourse.bass as bass
import concourse.tile as tile
from concourse import bacc, mybir
from concourse.bass import ts, ds

F32 = mybir.dt.float32
BF16 = mybir.dt.bfloat16
FP16 = mybir.dt.float16
AF = mybir.ActivationFunctionType

B, T, C = 8, 2048, 384
H, DH = 6, 64
SCALE = DH ** -0.5
NCORES = 8
TJ = 512            # q-block width
NJ = T // TJ        # 4 q-blocks
SC = 128            # s-chunk
NCI = C // 128      # 3 channel chunks
NHP = H // 2        # 3 head pairs (= hd blocks)


def build_kernel():
    nc = bacc.Bacc("TRN2", target_bir_lowering=False, debug=False)

    xT_d = nc.dram_tensor("xT", [128, NCI, T], BF16, kind="ExternalInput").ap()
    wq_d = nc.dram_tensor("wq", [128, NCI, C], BF16, kind="ExternalInput").ap()
    wk_d = nc.dram_tensor("wk", [128, NCI, C], BF16, kind="ExternalInput").ap()
    wv_d = nc.dram_tensor("wv", [128, NCI, C], BF16, kind="ExternalInput").ap()
    wp_d = nc.dram_tensor("wp", [128, NCI, C], BF16, kind="ExternalInput").ap()
    biasb_d = nc.dram_tensor("biasb", [128, C], F32, kind="ExternalInput").ap()
    y_d = nc.dram_tensor("y", [T, C], F32, kind="ExternalOutput").ap()

    with tile.TileContext(nc) as tc:
        with tc.tile_pool(name="const", bufs=1) as cpool, \
             tc.tile_pool(name="pp", bufs=2) as pp, \
             tc.tile_pool(name="sp", bufs=2, space="PSUM") as sp, \
             tc.tile_pool(name="op", bufs=1, space="PSUM") as op, \
             tc.tile_pool(name="ax", bufs=1, space="PSUM") as ax:
            xT = cpool.tile([128, NCI, T], BF16)
            wq = cpool.tile([128, NCI, C], BF16)
            wk = cpool.tile([128, NCI, C], BF16)
            wv = cpool.tile([128, NCI, C], BF16)
            wp = cpool.tile([128, NCI, C], BF16)
            biasb = cpool.tile([128, C], F32)
            QT = cpool.tile([128, NCI, T], BF16)
            KT = cpool.tile([128, NCI, T], BF16)
            attT = cpool.tile([128, NCI, T], BF16)
            Vt = cpool.tile([128, 16, H, 65], BF16)
            ones16 = cpool.tile([1, 64], FP16)
            scr = cpool.tile([1, 64], F32)

            nc.gpsimd.memset(ones16[:], 1.0)
            # preload the exp activation table while DMAs run
            nc.scalar.activation(scr[:], ones16[:], AF.Exp, scale=1.0)
            # whole-tile memset; V copies overwrite cols 0:64 leaving
            # col 64 == 1.0 (denominator trick)
            nc.gpsimd.memset(Vt[:], 1.0)

            # halves of xT so the projection prefix starts on partial data
            nc.sync.dma_start(xT[:, 0, 0:1024], xT_d[:, 0, 0:1024])
            nc.sync.dma_start(wk[:], wk_d[:])
            nc.sync.dma_start(wq[:], wq_d[:])
            nc.sync.dma_start(xT[:, 1, 0:1024], xT_d[:, 1, 0:1024])
            nc.sync.dma_start(xT[:, 2, 0:1024], xT_d[:, 2, 0:1024])
            nc.sync.dma_start(wv[:], wv_d[:])
            nc.sync.dma_start(xT[:, 0, 1024:T], xT_d[:, 0, 1024:T])
            nc.sync.dma_start(xT[:, 1, 1024:T], xT_d[:, 1, 1024:T])
            nc.sync.dma_start(xT[:, 2, 1024:T], xT_d[:, 2, 1024:T])
            nc.sync.dma_start(wp[:], wp_d[:])
            nc.sync.dma_start(biasb[:], biasb_d[:])

            # zero-init both S psum buffers: wide exp calls read full tiles
            # and must never see boot garbage (NaN) even in unused columns
            for _ in range(2):
                z0 = sp.tile([128, 1024], F32, tag="S", name="z0")
                nc.vector.memset(z0[:], 0.0)

            # aux psum: two banks, round-robin
            aux_rr = [0]

            def aux_tile():
                tag = "WA" if aux_rr[0] == 0 else "WB"
                aux_rr[0] ^= 1
                return ax.tile([128, TJ], F32, tag=tag, name="aux")

            # ---- phase-1 projection tasks ----
            done = set()

            def proj_qk(dst, w, bi, q):  # one 512-col quarter
                ps = aux_tile()
                for ci in range(NCI):
                    nc.tensor.matmul(
                        ps[:],
                        lhsT=w[:, ci, ts(bi, 128)],
                        rhs=xT[:, ci, ts(q, 512)],
                        start=(ci == 0), stop=(ci == NCI - 1),
                    )
                nc.vector.tensor_copy(dst[:, bi, ts(q, 512)], ps[:])

            def proj_v(si):
                ps = aux_tile()
                for ci in range(NCI):
                    nc.tensor.matmul(
                        ps[:, 0:C],
                        lhsT=xT[:, ci, ts(si, 128)],
                        rhs=wv[:, ci, :],
                        start=(ci == 0), stop=(ci == NCI - 1),
                    )
                nc.vector.tensor_copy(
                    Vt[:, si, :, 0:64],
                    ps[:, 0:C].rearrange("p (h d) -> p h d", h=H),
                )

            def emit_key(key):
                if key in done:
                    return
                done.add(key)
                if key[0] == 'K':
                    proj_qk(KT, wk, key[1], key[2])
                elif key[0] == 'Q':
                    proj_qk(QT, wq, key[1], key[2])
                else:
                    proj_v(key[1])

            # prefix: exactly what (j=0, hp=0) starts with
            for key in [('K', 0, 0), ('Q', 0, 0), ('V', 0), ('V', 1)]:
                emit_key(key)

            # per-frame drip plan: frame f = 3*j + hp -> ordered task list,
            # consumed one per drip slot (j0: i in 1,2,3; else i in 1,2,4,6,7)
            PLAN = {
                0: [('V', 2), ('K', 1, 0), ('Q', 1, 0)],
                1: [('K', 2, 0), ('Q', 2, 0)],
                2: [('K', 0, 1), ('Q', 0, 1)],
                3: [('V', 4), ('V', 5), ('V', 6), ('V', 7), ('Q', 1, 1)],
                4: [('K', 1, 1), ('K', 2, 1), ('Q', 2, 1), ('V', 8), ('V', 9)],
                5: [('Q', 0, 2), ('K', 0, 2), ('V', 10), ('V', 11)],
                6: [('Q', 1, 2), ('K', 1, 2), ('Q', 0, 3), ('K', 0, 3)],
                7: [('Q', 2, 2), ('K', 2, 2), ('V', 12), ('V', 13)],
                8: [('Q', 1, 3), ('K', 1, 3), ('V', 14), ('V', 15)],
                9: [('Q', 2, 3), ('K', 2, 3)],
            }

            ytasks = []       # deferred output-projection closures
            norm_pre = []     # deferred recip+fp16 cast (DVE only)
            norm_post = []    # deferred broadcast+tensor_mul
            ndone = [0]       # completed normalizations

            def make_y(j_, tb_, tail=False):
                def run():
                    if tail and tb_ >= 2:  # spread tail Y over all free banks
                        Yt = sp.tile([128, 1024], F32, tag="S", name="Yt")
                        Y = Yt[:, 0:C]
                    else:
                        Y = aux_tile()[:, 0:C]
                    for bi in range(NCI):
                        nc.tensor.matmul(
                            Y,
                            lhsT=attT[:, bi, ts(4 * j_ + tb_, 128)],
                            rhs=wp[:, bi, :],
                            start=(bi == 0), stop=(bi == NCI - 1),
                        )
                    ysb = pp.tile([128, C], F32, tag="ysb")
                    nc.vector.tensor_add(ysb[:], Y, biasb[:])
                    nc.sync.dma_start(y_d[ts(4 * j_ + tb_, 128), :], ysb[:])
                return run

            # ---- phase 2: attention, flat chunk stream with cross-frame
            # S-priming (each chunk's S-pair emitted one iteration ahead, so
            # the next frame's exps never wait on the previous frame's last
            # PV or eviction chain) ----
            chunks = [(j, hp, i, 4 * j + 4)
                      for j in range(NJ) for hp in range(NHP)
                      for i in range(4 * j + 4)]
            Ps = {}
            frame = {}  # current PV-side frame state

            def emit_S(j, hp, i, nch):
                if i == 0:  # JIT safety net for the new frame's K/Q
                    for q in range(j + 1):
                        emit_key(('K', hp, q))
                    emit_key(('Q', hp, j))
                d = max(0, SC * i - TJ * j)
                st = sp.tile([128, 1024], F32, tag="S")
                for z in (0, 64):
                    nc.tensor.matmul(
                        st[:, 8 * z + d:8 * z + 512],
                        lhsT=KT[z:z + 64, hp, ts(i, SC)],
                        rhs=QT[z:z + 64, hp, ds(TJ * j + d, TJ - d)],
                        start=True, stop=True,
                    )
                P = pp.tile([128, 1024], BF16, tag="P", bufs=3)
                if d <= 128:
                    nc.scalar.activation(P[:], st[:], AF.Exp, scale=SCALE)
                else:
                    for z in (0, 512):
                        nc.scalar.activation(P[:, z + d:z + 512],
                                             st[:, z + d:z + 512],
                                             AF.Exp, scale=SCALE)
                if SC * i >= TJ * j:  # fringe: mask diag window
                    for z in (0, 512):
                        nc.gpsimd.affine_select(
                            out=P[:, z + d:z + d + 128],
                            in_=P[:, z + d:z + d + 128],
                            pattern=[[1, 128]],
                            compare_op=mybir.AluOpType.is_ge,
                            fill=0.0, base=0, channel_multiplier=-1,
                        )
                Ps[(j, hp, i)] = P

            def emit_PV(j, hp, i, nch):
                if i == 0:
                    frame['Oa'] = op.tile([65, TJ], F32, tag="Oa", name="Oa")
                    frame['Ob'] = op.tile([65, TJ], F32, tag="Ob", name="Ob")
                d = max(0, SC * i - TJ * j)
                emit_key(('V', i))
                P = Ps.pop((j, hp, i))
                for O, z, h in ((frame['Oa'], 0, 2 * hp),
                                (frame['Ob'], 512, 2 * hp + 1)):
                    nc.tensor.matmul(
                        O[:, d:TJ],
                        lhsT=Vt[:, i, h, :],
                        rhs=P[:, z + d:z + 512],
                        start=(i == 0), stop=(i == nch - 1),
                    )

            def make_pre(dsb_):
                r16 = pp.tile([1, 2 * TJ], FP16, tag="r16", name="r16")
                def run():
                    rr = pp.tile([1, 2 * TJ], F32, tag="rr")
                    nc.vector.reciprocal_approx_fast(rr[:], dsb_[:])
                    nc.vector.tensor_copy(r16[:], rr[:])
                return run, r16

            def make_post(j_, hp_, r16_):
                def run():
                    rB = aux_tile()
                    for z in (0, 64):
                        for q in range(4):
                            nc.tensor.matmul(
                                rB[z:z + 64, ts(q, 128)],
                                lhsT=ones16[0:1, :],
                                rhs=r16_[0:1, ds(8 * z + 128 * q, 128)],
                                start=True, stop=True,
                            )
                    nc.vector.tensor_mul(attT[:, hp_, ts(j_, TJ)],
                                         attT[:, hp_, ts(j_, TJ)],
                                         rB[:])
                    ndone[0] += 1
                return run

            plan = []
            emit_S(*chunks[0])
            emit_S(*chunks[1])
            for k, (j, hp, i, nch) in enumerate(chunks):
                if k + 2 < len(chunks):
                    emit_S(*chunks[k + 2])
                emit_PV(j, hp, i, nch)
                if i == 0:
                    plan = list(PLAN.get(3 * j + hp, []))
                norm_slot = 3 if j == 0 else 5
                drip_slots = (1, 2, 3) if j == 0 else (1, 2, 4, 6, 7)
                if i == 1 and norm_pre:
                    norm_pre.pop(0)()
                if i in drip_slots and plan:
                    emit_key(plan.pop(0))
                elif i == norm_slot and norm_post:
                    norm_post.pop(0)()
                elif i >= 8 and i % 2 == 0 and ytasks and \
                        ndone[0] >= 3 * (ytasks[0][0] + 1):
                    ytasks.pop(0)[1]()
                if i == nch - 1:
                    # evict attT (unnormalized) + denominator rows (a first:
                    # the next frame's first PV waits on Oa's readers)
                    Oa, Ob = frame['Oa'], frame['Ob']
                    dsb = pp.tile([1, 2 * TJ], F32, tag="dsb")
                    nc.vector.tensor_copy(attT[0:64, hp, ts(j, TJ)],
                                          Oa[0:64, :])
                    nc.vector.tensor_copy(dsb[0:1, 0:TJ], Oa[64:65, :])
                    nc.vector.tensor_copy(attT[64:128, hp, ts(j, TJ)],
                                          Ob[0:64, :])
                    nc.vector.tensor_copy(dsb[0:1, TJ:2 * TJ], Ob[64:65, :])
                    pre, r16h = make_pre(dsb)
                    norm_pre.append(pre)
                    norm_post.append(make_post(j, hp, r16h))
                    if hp == NHP - 1:
                        for tb in range(4):
                            ytasks.append(
                                (j, make_y(j, tb, tail=(j == NJ - 1))))

            while norm_pre:   # tail: last normalize + j=3 output blocks
                norm_pre.pop(0)()
            while norm_post:
                norm_post.pop(0)()
            while ytasks:
                ytasks.pop(0)[1]()

    nc.compile()
    return nc


def _prep_inputs(x, Wq, Wk, Wv, Wp, bp):
    """Host-side shard + layout prep. Returns per-core input maps."""
    bf = ml_dtypes.bfloat16
    x = np.asarray(x, dtype=np.float32)

    def pack_w(W):  # [H, C, Dh] -> [128, NCI, H*Dh]
        Whd = np.transpose(np.asarray(W, np.float32), (1, 0, 2)).reshape(C, H * DH)
        return np.ascontiguousarray(
            Whd.reshape(NCI, 128, H * DH).transpose(1, 0, 2)
        ).astype(bf)

    wq_p, wk_p, wv_p = pack_w(Wq), pack_w(Wk), pack_w(Wv)
    wp_p = np.ascontiguousarray(
        np.asarray(Wp, np.float32).reshape(NCI, 128, C).transpose(1, 0, 2)
    ).astype(bf)
    biasb = np.broadcast_to(np.asarray(bp, np.float32), (128, C)).copy()

    in_maps = []
    for b in range(B):
        xT = np.ascontiguousarray(
            x[b].T.reshape(NCI, 128, T).transpose(1, 0, 2)
        ).astype(bf)
        in_maps.append({
            "xT": xT, "wq": wq_p, "wk": wk_p, "wv": wv_p, "wp": wp_p,
            "biasb": biasb,
        })
    return in_maps


_CACHE = {}


def kernel(x, Wq, Wk, Wv, Wp, bp):
    from concourse.bass_utils import run_bass_kernel_spmd

    if "nc" not in _CACHE:
        _CACHE["nc"] = build_kernel()
    nc = _CACHE["nc"]
    in_maps = _prep_inputs(x, Wq, Wk, Wv, Wp, bp)
    res = run_bass_kernel_spmd(nc, in_maps, list(range(NCORES)))
    out = np.stack([res.results[b]["y"] for b in range(B)], axis=0)
    return out.astype(np.float32)


# revision 17
# speedup vs baseline: 1.2996x; 1.0035x over previous
"""Multi-head causal attention (B=8, T=2048, C=384, H=6, Dh=64) on 8 TRN2 cores.

Sharding: data-parallel over batch - core b computes batch element b end to end
(no collectives).

v5 layout (all "T" means transposed, head-dim/channel on partitions):
  xT   [128, 3, 2048]  bf16   c = 128*ci + p
  wq/wk[128, 3, 384]   bf16   packed Wq[h,c,d] -> [c, h*64+d]
  wv/wp[128, 3, 384]   bf16
  biasb[128, 384]      f32    bias replicated across partitions

Per-core compute:
  QT/KT [hd, t] via matmul; Vt [s, h, 65] augmented (col 64 == 1 -> denom row).
  Attention per (j q-block of 512, hp head-pair, i s-chunk of 128):
    S-pair [128, 1024] = two concurrent K=64 matmuls (tile rows 0:64 / 64:128)
    one wide exp (ACT) -> P bf16 [128, 1024]; causal diag via affine_select
    PV per head accumulates O[65, 512] (row 64 = softmax denominator)
  PSUM: 4 banks pure S rotation (2x [128,1024], nothing else touches it until
  the tail), 2 banks O (Oa/Ob), 2 banks aux (round-robin for projection
  drip tasks, recipB broadcasts, and Y accumulations).
  Denominator chain is split + deferred: denom rows -> (next frame i==1)
  reciprocal_approx_fast + fp16 cast -> (i==5) K=1 broadcast matmuls into an
  aux recipB tile -> tensor_mul normalizes attT in place.  Output projection
  is a dense K=128 accumulation Y[t,e] = attT^T @ wp (+bias on eviction),
  scheduled into late chunk slots; the last q-block's Y runs on the freed S
  banks at the tail.  Phase-1 projections follow a per-frame slot plan with
  JIT fallback.
"""

import numpy as np
import ml_dtypes

import concourse.bass as bass
import concourse.tile as tile
from concourse import bacc, mybir
from concourse.bass import ts, ds

F32 = mybir.dt.float32
BF16 = mybir.dt.bfloat16
FP16 = mybir.dt.float16
AF = mybir.ActivationFunctionType

B, T, C = 8, 2048, 384
H, DH = 6, 64
SCALE = DH ** -0.5
NCORES = 8
TJ = 512            # q-block width
NJ = T // TJ        # 4 q-blocks
SC = 128            # s-chunk
NCI = C // 128      # 3 channel chunks
NHP = H // 2        # 3 head pairs (= hd blocks)


def build_kernel():
    nc = bacc.Bacc("TRN2", target_bir_lowering=False, debug=False)

    xT_d = nc.dram_tensor("xT", [128, NCI, T], BF16, kind="ExternalInput").ap()
    wq_d = nc.dram_tensor("wq", [128, NCI, C], BF16, kind="ExternalInput").ap()
    wk_d = nc.dram_tensor("wk", [128, NCI, C], BF16, kind="ExternalInput").ap()
    wv_d = nc.dram_tensor("wv", [128, NCI, C], BF16, kind="ExternalInput").ap()
    wp_d = nc.dram_tensor("wp", [128, NCI, C], BF16, kind="ExternalInput").ap()
    biasb_d = nc.dram_tensor("biasb", [128, C], F32, kind="ExternalInput").ap()
    y_d = nc.dram_tensor("y", [T, C], F32, kind="ExternalOutput").ap()

    with tile.TileContext(nc) as tc:
        with tc.tile_pool(name="const", bufs=1) as cpool, \
             tc.tile_pool(name="pp", bufs=2) as pp, \
             tc.tile_pool(name="sp", bufs=2, space="PSUM") as sp, \
             tc.tile_pool(name="op", bufs=1, space="PSUM") as op, \
             tc.tile_pool(name="ax", bufs=1, space="PSUM") as ax:
            xT = cpool.tile([128, NCI, T], BF16)
            wq = cpool.tile([128, NCI, C], BF16)
            wk = cpool.tile([128, NCI, C], BF16)
            wv = cpool.tile([128, NCI, C], BF16)
            wp = cpool.tile([128, NCI, C], BF16)
            biasb = cpool.tile([128, C], F32)
            QT = cpool.tile([128, NCI, T], BF16)
            KT = cpool.tile([128, NCI, T], BF16)
            attT = cpool.tile([128, NCI, T], BF16)
            Vt = cpool.tile([128, 16, H, 65], BF16)
            ones16 = cpool.tile([1, 64], FP16)
            scr = cpool.tile([1, 64], F32)

            nc.gpsimd.memset(ones16[:], 1.0)
            # preload the exp activation table while DMAs run
            nc.scalar.activation(scr[:], ones16[:], AF.Exp, scale=1.0)
            # whole-tile memset; V copies overwrite cols 0:64 leaving
            # col 64 == 1.0 (denominator trick)
            nc.gpsimd.memset(Vt[:], 1.0)

            # halves of xT so the projection prefix starts on partial data
            nc.sync.dma_start(xT[:, 0, 0:1024], xT_d[:, 0, 0:1024])
            nc.sync.dma_start(wk[:], wk_d[:])
            nc.sync.dma_start(wq[:], wq_d[:])
            nc.sync.dma_start(xT[:, 1, 0:1024], xT_d[:, 1, 0:1024])
            nc.sync.dma_start(xT[:, 2, 0:1024], xT_d[:, 2, 0:1024])
            nc.sync.dma_start(wv[:], wv_d[:])
            nc.sync.dma_start(xT[:, 0, 1024:T], xT_d[:, 0, 1024:T])
            nc.sync.dma_start(xT[:, 1, 1024:T], xT_d[:, 1, 1024:T])
            nc.sync.dma_start(xT[:, 2, 1024:T], xT_d[:, 2, 1024:T])
            nc.sync.dma_start(wp[:], wp_d[:])
            nc.sync.dma_start(biasb[:], biasb_d[:])

            # zero-init both S psum buffers: wide exp calls read full tiles
            # and must never see boot garbage (NaN) even in unused columns
            for _ in range(2):
                z0 = sp.tile([128, 1024], F32, tag="S", name="z0")
                nc.vector.memset(z0[:], 0.0)

            # aux psum: two banks, round-robin
            aux_rr = [0]

            def aux_tile():
                tag = "WA" if aux_rr[0] == 0 else "WB"
                aux_rr[0] ^= 1
                return ax.tile([128, TJ], F32, tag=tag, name="aux")

            # ---- phase-1 projection tasks ----
            done = set()

            def proj_qk(dst, w, bi, q):  # one 512-col quarter
                ps = aux_tile()
                for ci in range(NCI):
                    nc.tensor.matmul(
                        ps[:],
                        lhsT=w[:, ci, ts(bi, 128)],
                        rhs=xT[:, ci, ts(q, 512)],
                        start=(ci == 0), stop=(ci == NCI - 1),
                    )
                nc.vector.tensor_copy(dst[:, bi, ts(q, 512)], ps[:])

            def proj_v(si):
                ps = aux_tile()
                for ci in range(NCI):
                    nc.tensor.matmul(
                        ps[:, 0:C],
                        lhsT=xT[:, ci, ts(si, 128)],
                        rhs=wv[:, ci, :],
                        start=(ci == 0), stop=(ci == NCI - 1),
                    )
                nc.vector.tensor_copy(
                    Vt[:, si, :, 0:64],
                    ps[:, 0:C].rearrange("p (h d) -> p h d", h=H),
                )

            def emit_key(key):
                if key in done:
                    return
                done.add(key)
                if key[0] == 'K':
                    proj_qk(KT, wk, key[1], key[2])
                elif key[0] == 'Q':
                    proj_qk(QT, wq, key[1], key[2])
                else:
                    proj_v(key[1])

            # prefix: exactly what (j=0, hp=0) starts with
            for key in [('K', 0, 0), ('Q', 0, 0), ('V', 0), ('V', 1)]:
                emit_key(key)

            # per-frame drip plan: frame f = 3*j + hp -> ordered task list,
            # consumed one per drip slot (j0: i in 1,2,3; else i in 1,2,4,6,7)
            PLAN = {
                0: [('V', 2), ('K', 1, 0), ('Q', 1, 0)],
                1: [('K', 2, 0), ('Q', 2, 0)],
                2: [('K', 0, 1), ('Q', 0, 1)],
                3: [('V', 4), ('V', 5), ('V', 6), ('V', 7), ('Q', 1, 1)],
                4: [('K', 1, 1), ('K', 2, 1), ('Q', 2, 1), ('V', 8), ('V', 9)],
                5: [('Q', 0, 2), ('K', 0, 2), ('V', 10), ('V', 11)],
                6: [('Q', 1, 2), ('K', 1, 2), ('Q', 0, 3), ('K', 0, 3)],
                7: [('Q', 2, 2), ('K', 2, 2), ('V', 12), ('V', 13)],
                8: [('Q', 1, 3), ('K', 1, 3), ('V', 14), ('V', 15)],
                9: [('Q', 2, 3), ('K', 2, 3)],
            }

            ytasks = []       # deferred output-projection closures
            norm_pre = []     # deferred recip+fp16 cast (DVE only)
            norm_post = []    # deferred broadcast+tensor_mul
            ndone = [0]       # completed normalizations

            def make_y(j_, tb_, tail=False):
                def run():
                    if tail and tb_ >= 2:  # spread tail Y over all free banks
                        Yt = sp.tile([128, 1024], F32, tag="S", name="Yt")
                        Y = Yt[:, 0:C]
                    else:
                        Y = aux_tile()[:, 0:C]
                    for bi in range(NCI):
                        nc.tensor.matmul(
                            Y,
                            lhsT=attT[:, bi, ts(4 * j_ + tb_, 128)],
                            rhs=wp[:, bi, :],
                            start=(bi == 0), stop=(bi == NCI - 1),
                        )
                    ysb = pp.tile([128, C], F32, tag="ysb")
                    nc.vector.tensor_add(ysb[:], Y, biasb[:])
                    nc.sync.dma_start(y_d[ts(4 * j_ + tb_, 128), :], ysb[:])
                return run

            # ---- phase 2: attention, flat chunk stream with cross-frame
            # S-priming (each chunk's S-pair emitted one iteration ahead, so
            # the next frame's exps never wait on the previous frame's last
            # PV or eviction chain) ----
            chunks = [(j, hp, i, 4 * j + 4)
                      for j in range(NJ) for hp in range(NHP)
                      for i in range(4 * j + 4)]
            Ps = {}
            frame = {}  # current PV-side frame state

            def emit_S(j, hp, i, nch):
                if i == 0:  # JIT safety net for the new frame's K/Q
                    for q in range(j + 1):
                        emit_key(('K', hp, q))
                    emit_key(('Q', hp, j))
                d = max(0, SC * i - TJ * j)
                st = sp.tile([128, 1024], F32, tag="S")
                for z in (0, 64):
                    nc.tensor.matmul(
                        st[:, 8 * z + d:8 * z + 512],
                        lhsT=KT[z:z + 64, hp, ts(i, SC)],
                        rhs=QT[z:z + 64, hp, ds(TJ * j + d, TJ - d)],
                        start=True, stop=True,
                    )
                P = pp.tile([128, 1024], BF16, tag="P", bufs=3)
                if d <= 128:
                    nc.scalar.activation(P[:], st[:], AF.Exp, scale=SCALE)
                else:
                    for z in (0, 512):
                        nc.scalar.activation(P[:, z + d:z + 512],
                                             st[:, z + d:z + 512],
                                             AF.Exp, scale=SCALE)
                if SC * i >= TJ * j:  # fringe: mask diag window
                    for z in (0, 512):
                        nc.gpsimd.affine_select(
                            out=P[:, z + d:z + d + 128],
                            in_=P[:, z + d:z + d + 128],
                            pattern=[[1, 128]],
                            compare_op=mybir.AluOpType.is_ge,
                            fill=0.0, base=0, channel_multiplier=-1,
                        )
                Ps[(j, hp, i)] = P

            def emit_PV(j, hp, i, nch):
                if i == 0:
                    frame['Oa'] = op.tile([65, TJ], F32, tag="Oa", name="Oa")
                    frame['Ob'] = op.tile([65, TJ], F32, tag="Ob", name="Ob")
                d = max(0, SC * i - TJ * j)
                emit_key(('V', i))
                P = Ps.pop((j, hp, i))
                for O, z, h in ((frame['Oa'], 0, 2 * hp),
                                (frame['Ob'], 512, 2 * hp + 1)):
                    nc.tensor.matmul(
                        O[:, d:TJ],
                        lhsT=Vt[:, i, h, :],
                        rhs=P[:, z + d:z + 512],
                        start=(i == 0), stop=(i == nch - 1),
                    )

            def make_pre(dsb_):
                r16 = pp.tile([1, 2 * TJ], FP16, tag="r16", name="r16")
                def run():
                    rr = pp.tile([1, 2 * TJ], F32, tag="rr")
                    nc.vector.reciprocal_approx_fast(rr[:], dsb_[:])
                    nc.vector.tensor_copy(r16[:], rr[:])
                return run, r16

            def make_post(j_, hp_, r16_):
                def run():
                    rB = aux_tile()
                    for z in (0, 64):
                        for q in range(4):
                            nc.tensor.matmul(
                                rB[z:z + 64, ts(q, 128)],
                                lhsT=ones16[0:1, :],
                                rhs=r16_[0:1, ds(8 * z + 128 * q, 128)],
                                start=True, stop=True,
                            )
                    nc.vector.tensor_mul(attT[:, hp_, ts(j_, TJ)],
                                         attT[:, hp_, ts(j_, TJ)],
                                         rB[:])
                    ndone[0] += 1
                return run

            plan = []
            emit_S(*chunks[0])
            emit_S(*chunks[1])
            for k, (j, hp, i, nch) in enumerate(chunks):
                if k + 2 < len(chunks):
                    emit_S(*chunks[k + 2])
                emit_PV(j, hp, i, nch)
                if i == 0:
                    plan = list(PLAN.get(3 * j + hp, []))
                norm_slot = 3 if j == 0 else 5
                drip_slots = (1, 2, 3) if j == 0 else (1, 2, 4, 6, 7)
                if i == 1 and norm_pre:
                    norm_pre.pop(0)()
                if i in drip_slots and plan:
                    emit_key(plan.pop(0))
                elif i == norm_slot and norm_post:
                    norm_post.pop(0)()
                elif i >= 8 and i % 2 == 0 and ytasks and \
                        ndone[0] >= 3 * (ytasks[0][0] + 1):
                    ytasks.pop(0)[1]()
                if i == nch - 1:
                    # evict attT (unnormalized) + denominator rows (a first:
                    # the next frame's first PV waits on Oa's readers)
                    Oa, Ob = frame['Oa'], frame['Ob']
                    dsb = pp.tile([1, 2 * TJ], F32, tag="dsb")
                    nc.vector.tensor_copy(attT[0:64, hp, ts(j, TJ)],
                                          Oa[0:64, :])
                    nc.vector.tensor_copy(dsb[0:1, 0:TJ], Oa[64:65, :])
                    nc.vector.tensor_copy(attT[64:128, hp, ts(j, TJ)],
                                          Ob[0:64, :])
                    nc.vector.tensor_copy(dsb[0:1, TJ:2 * TJ], Ob[64:65, :])
                    pre, r16h = make_pre(dsb)
                    norm_pre.append(pre)
                    norm_post.append(make_post(j, hp, r16h))
                    if hp == NHP - 1:
                        for tb in range(4):
                            ytasks.append(
                                (j, make_y(j, tb, tail=(j == NJ - 1))))

            while norm_pre:   # tail: last normalize + j=3 output blocks
                norm_pre.pop(0)()
            while norm_post:
                norm_post.pop(0)()
            while ytasks:
                ytasks.pop(0)[1]()

    nc.compile()
    return nc


def _prep_inputs(x, Wq, Wk, Wv, Wp, bp):
    """Host-side shard + layout prep. Returns per-core input maps."""
    bf = ml_dtypes.bfloat16
    x = np.asarray(x, dtype=np.float32)

    def pack_w(W):  # [H, C, Dh] -> [128, NCI, H*Dh]
        Whd = np.transpose(np.asarray(W, np.float32), (1, 0, 2)).reshape(C, H * DH)
        return np.ascontiguousarray(
            Whd.reshape(NCI, 128, H * DH).transpose(1, 0, 2)
        ).astype(bf)

    wq_p, wk_p, wv_p = pack_w(Wq), pack_w(Wk), pack_w(Wv)
    wp_p = np.ascontiguousarray(
        np.asarray(Wp, np.float32).reshape(NCI, 128, C).transpose(1, 0, 2)
    ).astype(bf)
    biasb = np.broadcast_to(np.asarray(bp, np.float32), (128, C)).copy()

    in_maps = []
    for b in range(B):
        xT = np.ascontiguousarray(
            x[b].T.reshape(NCI, 128, T).transpose(1, 0, 2)
        ).astype(bf)
        in_maps.append({
            "xT": xT, "wq": wq_p, "wk": wk_p, "wv": wv_p, "wp": wp_p,
            "biasb": biasb,
        })
    return in_maps


_CACHE = {}


def kernel(x, Wq, Wk, Wv, Wp, bp):
    from concourse.bass_utils import run_bass_kernel_spmd

    if "nc" not in _CACHE:
        _CACHE["nc"] = build_kernel()
    nc = _CACHE["nc"]
    in_maps = _prep_inputs(x, Wq, Wk, Wv, Wp, bp)
    res = run_bass_kernel_spmd(nc, in_maps, list(range(NCORES)))
    out = np.stack([res.results[b]["y"] for b in range(B)], axis=0)
    return out.astype(np.float32)


# revision 18
# speedup vs baseline: 1.3375x; 1.0292x over previous
"""Multi-head causal attention (B=8, T=2048, C=384, H=6, Dh=64) on 8 TRN2 cores.

Sharding: data-parallel over batch - core b computes batch element b end to end
(no collectives).

v5 layout (all "T" means transposed, head-dim/channel on partitions):
  xT   [128, 3, 2048]  bf16   c = 128*ci + p
  wq/wk[128, 3, 384]   bf16   packed Wq[h,c,d] -> [c, h*64+d]
  wv/wp[128, 3, 384]   bf16
  biasb[128, 384]      f32    bias replicated across partitions

Per-core compute:
  QT/KT [hd, t] via matmul; Vt [s, h, 65] augmented (col 64 == 1 -> denom row).
  Attention per (j q-block of 512, hp head-pair, i s-chunk of 128):
    S-pair [128, 1024] = two concurrent K=64 matmuls (tile rows 0:64 / 64:128)
    one wide exp (ACT) -> P bf16 [128, 1024]; causal diag via affine_select
    PV per head accumulates O[65, 512] (row 64 = softmax denominator)
  PSUM: 4 banks pure S rotation (2x [128,1024], nothing else touches it until
  the tail), 2 banks O (Oa/Ob), 2 banks aux (round-robin for projection
  drip tasks, recipB broadcasts, and Y accumulations).
  Denominator chain is split + deferred: denom rows -> (next frame i==1)
  reciprocal_approx_fast + fp16 cast -> (i==5) K=1 broadcast matmuls into an
  aux recipB tile -> tensor_mul normalizes attT in place.  Output projection
  is a dense K=128 accumulation Y[t,e] = attT^T @ wp (+bias on eviction),
  scheduled into late chunk slots; the last q-block's Y runs on the freed S
  banks at the tail.  Phase-1 projections follow a per-frame slot plan with
  JIT fallback.
"""

import numpy as np
import ml_dtypes

import concourse.bass as bass
import concourse.tile as tile
from concourse import bacc, mybir
from concourse.bass import ts, ds

F32 = mybir.dt.float32
BF16 = mybir.dt.bfloat16
FP16 = mybir.dt.float16
AF = mybir.ActivationFunctionType

B, T, C = 8, 2048, 384
H, DH = 6, 64
SCALE = DH ** -0.5
NCORES = 8
TJ = 512            # q-block width
NJ = T // TJ        # 4 q-blocks
SC = 128            # s-chunk
NCI = C // 128      # 3 channel chunks
NHP = H // 2        # 3 head pairs (= hd blocks)


def build_kernel():
    nc = bacc.Bacc("TRN2", target_bir_lowering=False, debug=False)

    xT_d = nc.dram_tensor("xT", [128, NCI, T], BF16, kind="ExternalInput").ap()
    wq_d = nc.dram_tensor("wq", [128, NCI, C], BF16, kind="ExternalInput").ap()
    wk_d = nc.dram_tensor("wk", [128, NCI, C], BF16, kind="ExternalInput").ap()
    wv_d = nc.dram_tensor("wv", [128, NCI, C], BF16, kind="ExternalInput").ap()
    wp_d = nc.dram_tensor("wp", [128, NCI, C], BF16, kind="ExternalInput").ap()
    biasb_d = nc.dram_tensor("biasb", [128, C], F32, kind="ExternalInput").ap()
    y_d = nc.dram_tensor("y", [T, C], F32, kind="ExternalOutput").ap()

    with tile.TileContext(nc) as tc:
        with tc.tile_pool(name="const", bufs=1) as cpool, \
             tc.tile_pool(name="pp", bufs=2) as pp, \
             tc.tile_pool(name="sp", bufs=2, space="PSUM") as sp, \
             tc.tile_pool(name="op", bufs=1, space="PSUM") as op, \
             tc.tile_pool(name="ax", bufs=1, space="PSUM") as ax:
            xT = cpool.tile([128, NCI, T], BF16)
            wq = cpool.tile([128, NCI, C], BF16)
            wk = cpool.tile([128, NCI, C], BF16)
            wv = cpool.tile([128, NCI, C], BF16)
            wp = cpool.tile([128, NCI, C], BF16)
            biasb = cpool.tile([128, C], F32)
            QT = cpool.tile([128, NCI, T], BF16)
            KT = cpool.tile([128, NCI, T], BF16)
            attT = cpool.tile([128, NCI, T], BF16)
            Vt = cpool.tile([128, 16, H, 65], BF16)
            ones16 = cpool.tile([1, 64], FP16)
            scr = cpool.tile([1, 64], F32)

            nc.gpsimd.memset(ones16[:], 1.0)
            # preload the exp activation table while DMAs run
            nc.scalar.activation(scr[:], ones16[:], AF.Exp, scale=1.0)
            # whole-tile memset; V copies overwrite cols 0:64 leaving
            # col 64 == 1.0 (denominator trick)
            nc.gpsimd.memset(Vt[:], 1.0)

            # halves of xT so the projection prefix starts on partial data
            nc.sync.dma_start(xT[:, 0, 0:1024], xT_d[:, 0, 0:1024])
            nc.sync.dma_start(wk[:], wk_d[:])
            nc.sync.dma_start(wq[:], wq_d[:])
            nc.sync.dma_start(xT[:, 1, 0:1024], xT_d[:, 1, 0:1024])
            nc.sync.dma_start(xT[:, 2, 0:1024], xT_d[:, 2, 0:1024])
            nc.sync.dma_start(wv[:], wv_d[:])
            nc.sync.dma_start(xT[:, 0, 1024:T], xT_d[:, 0, 1024:T])
            nc.sync.dma_start(xT[:, 1, 1024:T], xT_d[:, 1, 1024:T])
            nc.sync.dma_start(xT[:, 2, 1024:T], xT_d[:, 2, 1024:T])
            nc.sync.dma_start(wp[:], wp_d[:])
            nc.sync.dma_start(biasb[:], biasb_d[:])

            # zero-init both S psum buffers: wide exp calls read full tiles
            # and must never see boot garbage (NaN) even in unused columns
            for _ in range(2):
                z0 = sp.tile([128, 1024], F32, tag="S", name="z0")
                nc.vector.memset(z0[:], 0.0)

            # aux psum: two banks, round-robin
            aux_rr = [0]

            def aux_tile():
                tag = "WA" if aux_rr[0] == 0 else "WB"
                aux_rr[0] ^= 1
                return ax.tile([128, TJ], F32, tag=tag, name="aux")

            # ---- phase-1 projection tasks ----
            done = set()

            def proj_qk(dst, w, bi, q):  # one 512-col quarter
                ps = aux_tile()
                for ci in range(NCI):
                    nc.tensor.matmul(
                        ps[:],
                        lhsT=w[:, ci, ts(bi, 128)],
                        rhs=xT[:, ci, ts(q, 512)],
                        start=(ci == 0), stop=(ci == NCI - 1),
                    )
                nc.vector.tensor_copy(dst[:, bi, ts(q, 512)], ps[:])

            def proj_v(si):
                ps = aux_tile()
                for ci in range(NCI):
                    nc.tensor.matmul(
                        ps[:, 0:C],
                        lhsT=xT[:, ci, ts(si, 128)],
                        rhs=wv[:, ci, :],
                        start=(ci == 0), stop=(ci == NCI - 1),
                    )
                nc.vector.tensor_copy(
                    Vt[:, si, :, 0:64],
                    ps[:, 0:C].rearrange("p (h d) -> p h d", h=H),
                )

            def emit_key(key):
                if key in done:
                    return
                done.add(key)
                if key[0] == 'K':
                    proj_qk(KT, wk, key[1], key[2])
                elif key[0] == 'Q':
                    proj_qk(QT, wq, key[1], key[2])
                else:
                    proj_v(key[1])

            # prefix: exactly what (j=0, hp=0) starts with
            for key in [('K', 0, 0), ('Q', 0, 0), ('V', 0), ('V', 1)]:
                emit_key(key)

            # per-frame drip plan: frame f = 3*j + hp -> ordered task list,
            # consumed one per drip slot (j0: i in 1,2,3; else i in 1,2,4,6,7)
            PLAN = {
                0: [('V', 2), ('K', 1, 0), ('Q', 1, 0)],
                1: [('K', 2, 0), ('Q', 2, 0)],
                2: [('K', 0, 1), ('Q', 0, 1)],
                3: [('V', 4), ('V', 5), ('V', 6), ('V', 7), ('Q', 1, 1)],
                4: [('K', 1, 1), ('K', 2, 1), ('Q', 2, 1), ('V', 8), ('V', 9)],
                5: [('Q', 0, 2), ('K', 0, 2), ('V', 10), ('V', 11)],
                6: [('Q', 1, 2), ('K', 1, 2), ('Q', 0, 3), ('K', 0, 3)],
                7: [('Q', 2, 2), ('K', 2, 2), ('V', 12), ('V', 13)],
                8: [('Q', 1, 3), ('K', 1, 3), ('V', 14), ('V', 15)],
                9: [('Q', 2, 3), ('K', 2, 3)],
            }

            ytasks = []       # deferred output-projection closures
            norm_pre = []     # deferred recip+fp16 cast (DVE only)
            norm_post = []    # deferred broadcast+tensor_mul
            ndone = [0]       # completed normalizations

            def make_y(j_, tb_, tail=False):
                def run():
                    if tail and tb_ >= 2:  # spread tail Y over all free banks
                        Yt = sp.tile([128, 1024], F32, tag="S", name="Yt")
                        Y = Yt[:, 0:C]
                    else:
                        Y = aux_tile()[:, 0:C]
                    for bi in range(NCI):
                        nc.tensor.matmul(
                            Y,
                            lhsT=attT[:, bi, ts(4 * j_ + tb_, 128)],
                            rhs=wp[:, bi, :],
                            start=(bi == 0), stop=(bi == NCI - 1),
                        )
                    ysb = pp.tile([128, C], F32, tag="ysb")
                    nc.vector.tensor_add(ysb[:], Y, biasb[:])
                    nc.sync.dma_start(y_d[ts(4 * j_ + tb_, 128), :], ysb[:])
                return run

            # ---- phase 2: attention, flat chunk stream with cross-frame
            # S-priming (each chunk's S-pair emitted one iteration ahead, so
            # the next frame's exps never wait on the previous frame's last
            # PV or eviction chain) ----
            chunks = [(j, hp, i, 4 * j + 4)
                      for j in range(NJ) for hp in range(NHP)
                      for i in range(4 * j + 4)]
            Ps = {}
            frame = {}  # current PV-side frame state

            def emit_S(j, hp, i, nch):
                if i == 0:  # JIT safety net for the new frame's K/Q
                    for q in range(j + 1):
                        emit_key(('K', hp, q))
                    emit_key(('Q', hp, j))
                d = max(0, SC * i - TJ * j)
                st = sp.tile([128, 1024], F32, tag="S")
                for z in (0, 64):
                    nc.tensor.matmul(
                        st[:, 8 * z + d:8 * z + 512],
                        lhsT=KT[z:z + 64, hp, ts(i, SC)],
                        rhs=QT[z:z + 64, hp, ds(TJ * j + d, TJ - d)],
                        start=True, stop=True,
                    )
                P = pp.tile([128, 1024], BF16, tag="P", bufs=4)
                if d <= 128:
                    nc.scalar.activation(P[:], st[:], AF.Exp, scale=SCALE)
                else:
                    for z in (0, 512):
                        nc.scalar.activation(P[:, z + d:z + 512],
                                             st[:, z + d:z + 512],
                                             AF.Exp, scale=SCALE)
                if SC * i >= TJ * j:  # fringe: mask diag window
                    for z in (0, 512):
                        nc.gpsimd.affine_select(
                            out=P[:, z + d:z + d + 128],
                            in_=P[:, z + d:z + d + 128],
                            pattern=[[1, 128]],
                            compare_op=mybir.AluOpType.is_ge,
                            fill=0.0, base=0, channel_multiplier=-1,
                        )
                Ps[(j, hp, i)] = P

            def emit_PV(j, hp, i, nch):
                if i == 0:
                    frame['Oa'] = op.tile([65, TJ], F32, tag="Oa", name="Oa")
                    frame['Ob'] = op.tile([65, TJ], F32, tag="Ob", name="Ob")
                d = max(0, SC * i - TJ * j)
                emit_key(('V', i))
                P = Ps.pop((j, hp, i))
                for O, z, h in ((frame['Oa'], 0, 2 * hp),
                                (frame['Ob'], 512, 2 * hp + 1)):
                    nc.tensor.matmul(
                        O[:, d:TJ],
                        lhsT=Vt[:, i, h, :],
                        rhs=P[:, z + d:z + 512],
                        start=(i == 0), stop=(i == nch - 1),
                    )

            def make_pre(dsb_):
                r16 = pp.tile([1, 2 * TJ], FP16, tag="r16", name="r16")
                def run():
                    rr = pp.tile([1, 2 * TJ], F32, tag="rr")
                    nc.vector.reciprocal_approx_fast(rr[:], dsb_[:])
                    nc.vector.tensor_copy(r16[:], rr[:])
                return run, r16

            def make_post(j_, hp_, r16_):
                def run():
                    rB = aux_tile()
                    for z in (0, 64):
                        for q in range(4):
                            nc.tensor.matmul(
                                rB[z:z + 64, ts(q, 128)],
                                lhsT=ones16[0:1, :],
                                rhs=r16_[0:1, ds(8 * z + 128 * q, 128)],
                                start=True, stop=True,
                            )
                    nc.vector.tensor_mul(attT[:, hp_, ts(j_, TJ)],
                                         attT[:, hp_, ts(j_, TJ)],
                                         rB[:])
                    ndone[0] += 1
                return run

            plan = []
            for c in chunks[0:3]:
                emit_S(*c)
            for k, (j, hp, i, nch) in enumerate(chunks):
                if k + 3 < len(chunks):
                    emit_S(*chunks[k + 3])
                emit_PV(j, hp, i, nch)
                if i == 0:
                    plan = list(PLAN.get(3 * j + hp, []))
                norm_slot = 3 if j == 0 else 5
                drip_slots = (1, 2, 3) if j == 0 else (1, 2, 4, 6, 7)
                if i == 1 and norm_pre:
                    norm_pre.pop(0)()
                if i in drip_slots and plan:
                    emit_key(plan.pop(0))
                elif i == norm_slot and norm_post:
                    norm_post.pop(0)()
                elif i >= 8 and i % 2 == 0 and ytasks and \
                        ndone[0] >= 3 * (ytasks[0][0] + 1):
                    ytasks.pop(0)[1]()
                if i == nch - 1:
                    # evict attT (unnormalized) + denominator rows (a first:
                    # the next frame's first PV waits on Oa's readers)
                    Oa, Ob = frame['Oa'], frame['Ob']
                    dsb = pp.tile([1, 2 * TJ], F32, tag="dsb")
                    nc.vector.tensor_copy(attT[0:64, hp, ts(j, TJ)],
                                          Oa[0:64, :])
                    nc.vector.tensor_copy(dsb[0:1, 0:TJ], Oa[64:65, :])
                    nc.vector.tensor_copy(attT[64:128, hp, ts(j, TJ)],
                                          Ob[0:64, :])
                    nc.vector.tensor_copy(dsb[0:1, TJ:2 * TJ], Ob[64:65, :])
                    pre, r16h = make_pre(dsb)
                    norm_pre.append(pre)
                    norm_post.append(make_post(j, hp, r16h))
                    if hp == NHP - 1:
                        for tb in range(4):
                            ytasks.append(
                                (j, make_y(j, tb, tail=(j == NJ - 1))))

            while norm_pre:   # tail: last normalize + j=3 output blocks
                norm_pre.pop(0)()
            while norm_post:
                norm_post.pop(0)()
            while ytasks:
                ytasks.pop(0)[1]()

    nc.compile()
    return nc


def _prep_inputs(x, Wq, Wk, Wv, Wp, bp):
    """Host-side shard + layout prep. Returns per-core input maps."""
    bf = ml_dtypes.bfloat16
    x = np.asarray(x, dtype=np.float32)

    def pack_w(W):  # [H, C, Dh] -> [128, NCI, H*Dh]
        Whd = np.transpose(np.asarray(W, np.float32), (1, 0, 2)).reshape(C, H * DH)
        return np.ascontiguousarray(
            Whd.reshape(NCI, 128, H * DH).transpose(1, 0, 2)
        ).astype(bf)

    wq_p, wk_p, wv_p = pack_w(Wq), pack_w(Wk), pack_w(Wv)
    wp_p = np.ascontiguousarray(
        np.asarray(Wp, np.float32).reshape(NCI, 128, C).transpose(1, 0, 2)
    ).astype(bf)
    biasb = np.broadcast_to(np.asarray(bp, np.float32), (128, C)).copy()

    in_maps = []
    for b in range(B):
        xT = np.ascontiguousarray(
            x[b].T.reshape(NCI, 128, T).transpose(1, 0, 2)
        ).astype(bf)
        in_maps.append({
            "xT": xT, "wq": wq_p, "wk": wk_p, "wv": wv_p, "wp": wp_p,
            "biasb": biasb,
        })
    return in_maps


_CACHE = {}


def kernel(x, Wq, Wk, Wv, Wp, bp):
    from concourse.bass_utils import run_bass_kernel_spmd

    if "nc" not in _CACHE:
        _CACHE["nc"] = build_kernel()
    nc = _CACHE["nc"]
    in_maps = _prep_inputs(x, Wq, Wk, Wv, Wp, bp)
    res = run_bass_kernel_spmd(nc, in_maps, list(range(NCORES)))
    out = np.stack([res.results[b]["y"] for b in range(B)], axis=0)
    return out.astype(np.float32)
